# revision 1
# baseline (speedup 1.0000x reference)
"""2-layer GCN (PyG GCNConv x2 + leaky_relu) on 8 Trainium2 NeuronCores.

v2 strategy (dst-partitioned, gather-free, For_i hardware loops):
  - Nodes partitioned 128-ALIGNED across 8 cores: core c owns padded ids
    [c*6272, c*6272+6250); padded table has NPAD=50176 rows (zeros in pads).
  - Normalization folded: table rows pre-scaled by dis[src]; dis[dst] applied
    post-aggregation (ACT scale). Self-loops NOT in the edge stream: handled
    by one identity matmul per dst tile reading the core's own rows
    (xown for layer 1, `part` for layer 2) — contributes dis_d^2 * x_d.
  - Edge routing has NO per-edge DMA. Two phases through a DRAM scratch M:
    Phase 1 (For_i over 392 src blocks): one-hot Sel (DVE is_equal vs iota)
      selects/duplicates rows of X_b [128,64] into bucket slots via PE
      matmul; 7 chunks/block -> stage [128,7*64] -> one contiguous DMA to
      M block region [896 rows, 64]. Block region row m holds slot
      (p=m//7, c=m%7); bucket (b,t) occupies rows m = t*16 + r (r<16).
    Phase 2 (For_i over 49 dst tiles): one strided DMA reads rows
      [16t,16t+16) of every block -> msb [128, 49*64] (slot q=b*16+r at
      partition q//49, col q%49); 49 one-hot scatter matmuls accumulate
      agg[128dst,64] in PSUM; + identity matmul (self loop) (+ rank-1
      disinv x b2 term closing layer 2's group).
  - Layer-1 tail per tile: ACT(dis) -> PE transpose -> W1 -> Lrelu+b1 ->
    W2 -> ACT(dis) -> part. One AllGather builds the layer-2 table.
  - Bucket capacity R = max bucket fill (>=16, 17 for this graph); M block
    slots padded to NBM=512 so CPT = 4R divides evenly for the strided DMA.
  - Routing path (tables, sel/scatter one-hots, M, exchange) runs in bf16
    (rel err ~1e-3, tol 2e-2); PSUM accumulation and the dense tail in f32.
  - One-hots are built in wide groups (broadcast tensor_tensor is_equal
    against iota) rather than per 128-slot chunk.

Self-contained: hardcodes shapes; compiles on first call keyed by edge hash.
"""

import os
import hashlib
import sys

import numpy as np

sys.path.insert(0, "/opt/trn_rl_repo")

# ---- problem constants ----
N, E = 50000, 800000
DIN, DH, DOUT = 64, 128, 64
P_CORES = 8
NP = N // P_CORES            # 6250 real nodes per core
NT = 49                      # dst tiles per core
NPP = NT * 128               # 6272 padded rows per core
NPAD = P_CORES * NPP         # 50176 padded table rows
NB = NPAD // 128             # 392 real src blocks (global)
NBM = 512                    # M-scratch block slots (pad => CPT = 4R exactly)
PAD = 200.0                  # one-hot miss value
NEG_SLOPE = 0.01


def _prep(edge_index: np.ndarray):
    src = np.asarray(edge_index[0], dtype=np.int64)
    dst = np.asarray(edge_index[1], dtype=np.int64)

    deg = (np.bincount(dst, minlength=N) + 1).astype(np.float32)
    dis = (1.0 / np.sqrt(deg)).astype(np.float32)

    pid_src = (src // NP) * NPP + (src % NP)      # padded id of src
    core = dst // NP
    tloc = (dst // NP) * 0 + (dst % NP)           # local dst 0..NP-1
    b_all = pid_src // 128                        # src block 0..NB-1
    t_all = tloc // 128                           # dst tile 0..NT-1
    srclo_all = pid_src % 128
    dstlo_all = tloc % 128

    # bucket ranks per (core, b, t)
    key = (core * NB + b_all) * NT + t_all
    order = np.argsort(key, kind="stable")
    ks = key[order]
    # rank within equal keys
    first = np.ones(len(ks), dtype=bool)
    first[1:] = ks[1:] != ks[:-1]
    starts = np.flatnonzero(first)
    run_id = np.cumsum(first) - 1
    r_sorted = np.arange(len(ks)) - starts[run_id]
    rmax = int(r_sorted.max()) + 1 if len(ks) else 1
    R = max(16, rmax)                             # bucket capacity
    SPB = ((NT * R + 127) // 128) * 128           # slots per block region
    CPB = SPB // 128                              # phase-1 chunks per block
    CPT = (NBM * R) // 128                        # phase-2 chunks per tile

    # srcloc: row vector of src-lo per slot, j = c*128 + p for slot at
    # physical row m = p*CPB + c (transposed one-hot built on device via
    # rank-1 PE broadcast + tensor_tensor is_equal)
    srcloc = np.full((P_CORES, 1, NBM * SPB), PAD, dtype=np.float32)
    dstloc = np.full((P_CORES, 128, NT * CPT), PAD, dtype=np.float32)

    co = core[order]
    bo = b_all[order]
    to = t_all[order]
    so = srclo_all[order]
    do = dstlo_all[order]
    # phase 1: block-region row m = t*R + r at (p=m//CPB, c=m%CPB)
    m = to * R + r_sorted
    p1 = m // CPB
    c1 = m % CPB
    srcloc[co, 0, bo * SPB + c1 * 128 + p1] = so.astype(np.float32)
    # phase 2: tile stream position q = b*R + r at (p=q//CPT, j=q%CPT)
    q = bo * R + r_sorted
    p2 = q // CPT
    j2 = q % CPT
    dstloc[co, p2, to * CPT + j2] = do.astype(np.float32)

    dis_t = np.zeros((P_CORES, 128, NT), dtype=np.float32)
    dinv = np.zeros((P_CORES, 1, NPP), dtype=np.float32)
    for c in range(P_CORES):
        d = dis[c * NP:(c + 1) * NP]
        pad = np.zeros(NPP, dtype=np.float32)
        pad[:NP] = d
        dis_t[c] = pad.reshape(NT, 128).T
        ipad = np.zeros(NPP, dtype=np.float32)
        ipad[:NP] = 1.0 / d
        dinv[c, 0] = ipad

    CPTR = (NB * R + 127) // 128   # chunks holding real (non-pad) blocks
    return dict(dis=dis, R=R, SPB=SPB, CPB=CPB, CPT=CPT, CPTR=CPTR,
                srcloc=srcloc, dstloc=dstloc, dis_t=dis_t, dinv=dinv)


# ---------------------------------------------------------------------------
# Bass kernel
# ---------------------------------------------------------------------------

def _build_nc(prep):
    import concourse.bass as bass
    import concourse.bacc as bacc
    import concourse.tile as tile
    from concourse import mybir

    f32 = mybir.dt.float32
    bf16 = mybir.dt.bfloat16
    AF = mybir.ActivationFunctionType
    ALU = mybir.AluOpType
    ds = bass.ds

    R, CPB, CPT, SPB = prep["R"], prep["CPB"], prep["CPT"], prep["SPB"]
    CPTR = prep["CPTR"]
    GB = 4                           # blocks per phase-1 loop iteration

    nc = bacc.Bacc(
        "TRN2", target_bir_lowering=False, debug=False,
        enable_asserts=False, num_devices=P_CORES,
    )

    H2 = SPB // 2
    xt_d = nc.dram_tensor("xt", [NPAD, DIN], bf16, kind="ExternalInput")
    xown_d = nc.dram_tensor("xown", [NPP, DIN], bf16, kind="ExternalInput")
    srcloc_d = nc.dram_tensor("srcloc", [1, NBM * SPB], f32,
                              kind="ExternalInput")
    ones1_d = nc.dram_tensor("ones1", [1, 128], f32, kind="ExternalInput")
    iotat_d = nc.dram_tensor("iotat", [128, H2], f32, kind="ExternalInput")
    dstloc_d = nc.dram_tensor("dstloc", [128, NT * CPT], f32,
                              kind="ExternalInput")
    dis_d = nc.dram_tensor("dis_t", [128, NT], f32, kind="ExternalInput")
    dinv_d = nc.dram_tensor("dinv", [1, NPP], f32, kind="ExternalInput")
    w1_d = nc.dram_tensor("w1", [DIN, DH], f32, kind="ExternalInput")
    w2_d = nc.dram_tensor("w2", [DH, DOUT], f32, kind="ExternalInput")
    b1_d = nc.dram_tensor("b1", [DH, 1], f32, kind="ExternalInput")
    b2_d = nc.dram_tensor("b2r", [1, DOUT], f32, kind="ExternalInput")
    iota_d = nc.dram_tensor("iota", [128, 128], f32, kind="ExternalInput")
    ident_d = nc.dram_tensor("ident", [128, 128], f32, kind="ExternalInput")
    identb_d = nc.dram_tensor("identb", [128, 128], bf16, kind="ExternalInput")
    out_d = nc.dram_tensor("outp", [NPP, DOUT], f32, kind="ExternalOutput")

    with tile.TileContext(nc) as tc:
        with (
            tc.tile_pool(name="const", bufs=1) as constp,
            tc.tile_pool(name="xb", bufs=3) as xpool,
            tc.tile_pool(name="stg", bufs=3) as stpool,
            tc.tile_pool(name="sl", bufs=3) as slpool,
            tc.tile_pool(name="sp", bufs=4) as spool,
            tc.tile_pool(name="msb", bufs=2) as mpool,
            tc.tile_pool(name="wk", bufs=2) as work,
            tc.tile_pool(name="p1", bufs=2, space="PSUM") as p1pool,
            tc.tile_pool(name="psel", bufs=1, space="PSUM") as pselp,
            tc.tile_pool(name="pagg", bufs=2, space="PSUM") as pagg,
            tc.tile_pool(name="ptr", bufs=1, space="PSUM") as ptr,
            tc.tile_pool(name="pg1", bufs=1, space="PSUM") as pg1,
            tc.tile_pool(name="pg2", bufs=1, space="PSUM") as pg2,
            tc.tile_pool(name="dram", bufs=1, space="DRAM") as dram,
        ):
            iota_sb = constp.tile([128, 128], f32)
            ident_sb = constp.tile([128, 128], f32)
            identb_sb = constp.tile([128, 128], bf16)
            ones1_sb = constp.tile([1, 128], f32)
            iotat_sb = constp.tile([128, H2], f32)
            w1_sb = constp.tile([DIN, DH], f32)
            w2_sb = constp.tile([DH, DOUT], f32)
            b1_sb = constp.tile([DH, 1], f32)
            b2_sb = constp.tile([1, DOUT], f32)
            for sb, dr in [(iota_sb, iota_d), (ident_sb, ident_d),
                           (identb_sb, identb_d),
                           (ones1_sb, ones1_d), (iotat_sb, iotat_d),
                           (w1_sb, w1_d), (w2_sb, w2_d),
                           (b1_sb, b1_d), (b2_sb, b2_d)]:
                nc.sync.dma_start(sb[:], dr[:])

            # M scratch lives across repeats; zero the pad-block regions
            # once (phase 2 reads them; dstloc=PAD keeps them out of sums,
            # but they must be finite).
            M_d = dram.tile([NBM, SPB * DIN], bf16, tag="M", bufs=1)
            zt = work.tile([128, SPB * DIN // 128], bf16, tag="zt")
            nc.gpsimd.memset(zt[:], 0.0)
            for b in range(NB, NBM):
                nc.sync.dma_start(M_d[b:b + 1, :], zt[:])

            for _rep in range(int(os.environ.get("GCN_REPEAT", "1"))):
                part = dram.tile([NPP, DOUT], bf16, tag="part", bufs=2)
                table = dram.tile([NPAD, DOUT], bf16, addr_space="Shared",
                                  tag="table", bufs=2)

                for lidx in range(2):
                    src_d = xt_d if lidx == 0 else table
                    own_d = xown_d if lidx == 0 else part
                    # ---- phase 1: route src blocks into bucket slots ----
                    with tc.For_i(0, NB // GB) as i:
                        slst = slpool.tile([1, GB * SPB], f32, tag="slst")
                        nc.sync.dma_start(
                            slst[:], srcloc_d[:, ds(i * (GB * SPB),
                                                    GB * SPB)])
                        for g in range(GB):
                            xb = xpool.tile([128, DIN], bf16, tag="xb")
                            nc.sync.dma_start(
                                xb[:],
                                src_d[ds(i * (GB * 128) + g * 128, 128), :])
                            # transposed one-hot:
                            # sel[s, j] = (s == srclo(slot j))
                            sel = spool.tile([128, SPB], bf16, tag="sel")
                            for h in range(2):
                                pr = pselp.tile([128, H2], f32, tag="pr")
                                nc.tensor.matmul(
                                    pr[:], ones1_sb[:],
                                    slst[:, g * SPB + h * H2:
                                         g * SPB + (h + 1) * H2],
                                    start=True, stop=True)
                                nc.vector.tensor_tensor(
                                    sel[:, h * H2:(h + 1) * H2],
                                    iotat_sb[:], pr[:], op=ALU.is_equal)
                            ps = p1pool.tile([128, CPB * DIN], f32,
                                             tag="ps")
                            for c in range(CPB):
                                nc.tensor.matmul(
                                    ps[:, c * DIN:(c + 1) * DIN],
                                    sel[:, c * 128:(c + 1) * 128], xb[:],
                                    start=True, stop=True)
                            stage = stpool.tile([128, CPB * DIN], bf16,
                                                tag="stage")
                            nc.scalar.activation(stage[:], ps[:], AF.Copy,
                                                 bias=0.0)
                            nc.sync.dma_start(
                                M_d[ds(i * GB + g, 1), :], stage[:])
                    # ---- phase 2: per dst tile aggregate + layer tail ----
                    with tc.For_i(0, NT // 7) as i:
                      for tt in range(7):
                        msb = mpool.tile([128, CPT * DIN], bf16, tag="msb")
                        nc.sync.dma_start(
                            msb[:],
                            M_d[:, ds(i * (7 * R * DIN) + tt * (R * DIN),
                                      R * DIN)].rearrange(
                                "b (r f) -> b r f", f=DIN))
                        xdg = xpool.tile([128, DIN], bf16, tag="xdg")
                        nc.sync.dma_start(
                            xdg[:],
                            own_d[ds(i * 896 + tt * 128, 128), :])
                        dlst = slpool.tile([128, CPT], f32, tag="dlst")
                        nc.sync.dma_start(
                            dlst[:],
                            dstloc_d[:, ds(i * (7 * CPT) + tt * CPT, CPT)])
                        dcol = slpool.tile([128, 1], f32, tag="dcol")
                        nc.sync.dma_start(dcol[:], dis_d[:, ds(i * 7 + tt, 1)])
                        # grouped one-hot build: 4 DVE ops cover CPT chunks
                        KG = CPT // 4
                        s_t = spool.tile([128, CPT * 128], bf16, tag="s2",
                                         bufs=2)
                        for g in range(4):
                            i0, i1 = bass.broadcast_tensor_aps(
                                iota_sb[:].rearrange(
                                    "p (one j) -> p one j", one=1),
                                dlst[:, g * KG:(g + 1) * KG].rearrange(
                                    "p (k one) -> p k one", one=1))
                            nc.vector.tensor_tensor(
                                s_t[:, g * KG * 128:(g + 1) * KG * 128]
                                .rearrange("p (k j) -> p k j", j=128),
                                i0, i1, op=ALU.is_equal)
                        agg = pagg.tile([128, DIN], f32, tag="agg")
                        for j in range(CPT):
                            nc.tensor.matmul(
                                agg[:], s_t[:, j * 128:(j + 1) * 128],
                                msb[:, j * DIN:(j + 1) * DIN],
                                start=(j == 0), stop=False)
                        nc.tensor.matmul(agg[:], identb_sb[:], xdg[:],
                                         start=False, stop=(lidx == 0))
                        if lidx == 0:
                            a_sb = work.tile([128, DIN], f32, tag="a_sb")
                            nc.scalar.activation(a_sb[:], agg[:], AF.Copy,
                                                 bias=0.0,
                                                 scale=dcol[:, 0:1])
                            aT_p = ptr.tile([DIN, 128], f32, tag="aT")
                            nc.tensor.transpose(aT_p[:], a_sb[:], ident_sb[:])
                            aT_sb = work.tile([DIN, 128], f32, tag="aT_sb")
                            nc.scalar.activation(aT_sb[:], aT_p[:], AF.Copy,
                                                 bias=0.0)
                            x1_p = pg1.tile([DH, 128], f32, tag="x1")
                            nc.tensor.matmul(x1_p[:], w1_sb[:], aT_sb[:],
                                             start=True, stop=True)
                            x1_sb = work.tile([DH, 128], f32, tag="x1_sb")
                            nc.scalar.activation(x1_sb[:], x1_p[:], AF.Lrelu,
                                                 bias=b1_sb[:, 0:1],
                                                 alpha=NEG_SLOPE)
                            x2_p = pg2.tile([128, DOUT], f32, tag="x2")
                            nc.tensor.matmul(x2_p[:], x1_sb[:], w2_sb[:],
                                             start=True, stop=True)
                            x2_sb = work.tile([128, DOUT], bf16, tag="x2_sb")
                            nc.scalar.activation(x2_sb[:], x2_p[:], AF.Copy,
                                                 bias=0.0,
                                                 scale=dcol[:, 0:1])
                            nc.sync.dma_start(
                                part[ds(i * 896 + tt * 128, 128), :],
                                x2_sb[:])
                        else:
                            div = slpool.tile([1, 128], f32, tag="div")
                            nc.sync.dma_start(
                                div[:],
                                dinv_d[:, ds(i * 896 + tt * 128, 128)])
                            nc.tensor.matmul(agg[:], div[:], b2_sb[:],
                                             start=False, stop=True)
                            o_sb = work.tile([128, DOUT], f32, tag="o_sb")
                            nc.scalar.activation(o_sb[:], agg[:], AF.Lrelu,
                                                 bias=0.0,
                                                 scale=dcol[:, 0:1],
                                                 alpha=NEG_SLOPE)
                            nc.sync.dma_start(
                                out_d[ds(i * 896 + tt * 128, 128), :],
                                o_sb[:])
                    if lidx == 0:
                        if os.environ.get("GCN_NOAG", "0") == "1":
                            # timing ablation: replace exchange with a local
                            # copy of this core's part (results are wrong)
                            nc.sync.dma_start(
                                table[0:NPP, :], part[:, :])
                        else:
                            nc.gpsimd.collective_compute(
                                "AllGather", mybir.AluOpType.bypass,
                                replica_groups=[list(range(P_CORES))],
                                ins=[part.opt()], outs=[table.opt()],
                            )

    nc.compile()
    return nc


def _make_in_maps(inputs, W1, b1, W2, b2, prep):
    import ml_dtypes
    dis = prep["dis"]
    xt = np.zeros((NPAD, DIN), dtype=np.float32)
    x32 = np.asarray(inputs, np.float32)
    for c in range(P_CORES):
        xt[c * NPP:c * NPP + NP] = (x32[c * NP:(c + 1) * NP]
                                    * dis[c * NP:(c + 1) * NP, None])
    xt = xt.astype(ml_dtypes.bfloat16)
    iota = np.tile(np.arange(128, dtype=np.float32), (128, 1))
    ident = np.eye(128, dtype=np.float32)
    in_maps = []
    for c in range(P_CORES):
        in_maps.append({
            "xt": xt,
            "xown": xt[c * NPP:(c + 1) * NPP],
            "srcloc": prep["srcloc"][c],
            "dstloc": prep["dstloc"][c],
            "dis_t": prep["dis_t"][c],
            "dinv": prep["dinv"][c],
            "w1": np.asarray(W1, np.float32),
            "w2": np.asarray(W2, np.float32),
            "b1": np.asarray(b1, np.float32).reshape(DH, 1),
            "b2r": np.asarray(b2, np.float32).reshape(1, DOUT),
            "iota": iota,
            "ident": ident,
            "identb": ident.astype(ml_dtypes.bfloat16),
            "ones1": np.ones((1, 128), dtype=np.float32),
            "iotat": np.tile(
                np.arange(128, dtype=np.float32)[:, None],
                (1, prep["SPB"] // 2)),
        })
    return in_maps


_CACHE = {}


def kernel(inputs, edge_index, W1, b1, W2, b2, _trace=False, _results_box=None):
    from concourse.bass_utils import run_bass_kernel_spmd

    edge_index = np.asarray(edge_index)
    key = hashlib.sha1(edge_index.tobytes()).hexdigest()
    key += ":r%s:n%s" % (os.environ.get("GCN_REPEAT", "1"),
                         os.environ.get("GCN_NOAG", "0"))
    if key not in _CACHE:
        prep = _prep(edge_index)
        nc = _build_nc(prep)
        _CACHE[key] = (prep, nc)
    prep, nc = _CACHE[key]
    in_maps = _make_in_maps(inputs, W1, b1, W2, b2, prep)
    res = run_bass_kernel_spmd(
        nc, in_maps, core_ids=list(range(P_CORES)), trace=_trace,
    )
    if _results_box is not None:
        _results_box.append(res)
    out = np.concatenate(
        [res.results[c]["outp"][:NP] for c in range(P_CORES)], axis=0
    )
    return out.astype(np.float32)



# revision 5
# speedup vs baseline: 3.9768x; 3.9768x over previous
"""2-layer GCN (PyG GCNConv x2 + leaky_relu) on 8 Trainium2 NeuronCores.

v3 strategy (gather + segment-reduce; tuned for the per-instruction-cost
regime of this stack):
  - Nodes are ranked by degree and dealt round-robin across the 8 cores
    (rank k -> core k%8, local slot k//8), so every core's tile t holds
    nodes of nearly identical degree. Per-tile slot capacities R_t are
    then tight and identical across cores (SPMD-uniform program).
  - Per dst tile (128 nodes): one dma_gather per int16-range half pulls
    all in-edge messages (and the self-loop row) into SBUF feature-major
    [128 feat, d*R + r]; a single DVE tensor_reduce per half does the
    segment sum; add merges halves. Tiles are processed in groups of 4
    (512 dst columns per instruction) to amortize the ~35us/instruction
    floor.
  - Tables are [rows, 128] bf16 (256B rows as dma_gather requires; cols
    64..127 are junk and never read). idxs are int16, so the table is
    split at row 32768 into lo/hi gathers; dummy slots point at
    guaranteed-zero pad rows.
  - Normalization: table rows pre-scaled by dis[src] (host for layer 1;
    the layer-1 tail emits dis^2 * lrelu(aggW1 + dinv b1) W2 so the
    AllGather'd table is already source-scaled). dis[dst] applied via
    precomputed [64, 6272] row-replicated factors (DVE multiply).
  - Layer tail runs feature-major: W1 matmul + rank-1 b1*dinv into PSUM,
    ACT lrelu, W2 matmul, DVE dis^2 scale, 4x PE transpose, ACT copy,
    one strided store per group. One AllGather builds the layer-2 table.

Self-contained: hardcodes shapes; compiles on first call keyed by edge hash.
"""

import os
import hashlib
import sys

import numpy as np

sys.path.insert(0, "/opt/trn_rl_repo")

# ---- problem constants ----
N, E = 50000, 800000
DIN, DH, DOUT = 64, 128, 64
P_CORES = 8
NP = N // P_CORES            # 6250 real nodes per core
NT = 49                      # dst tiles per core
NPP = NT * 128               # 6272 padded rows per core
NROWS = P_CORES * NPP        # 50176 table rows
SPLIT = 32768                # int16 gather index limit
GSZ = 4                      # tiles per group
NG = (NT + GSZ - 1) // GSZ   # 13 groups (12x4 + 1x1)
NEG_SLOPE = 0.01


def _prep(edge_index: np.ndarray):
    src = np.asarray(edge_index[0], dtype=np.int64)
    dst = np.asarray(edge_index[1], dtype=np.int64)

    deg = (np.bincount(dst, minlength=N) + 1).astype(np.float32)
    dis = (1.0 / np.sqrt(deg)).astype(np.float32)

    # degree-descending rank -> (core, local pos)
    rank_of = np.argsort(-deg, kind="stable")      # rank -> orig node
    newpos = np.empty(N, dtype=np.int64)           # orig -> rank
    newpos[rank_of] = np.arange(N)
    core_of = newpos % P_CORES                     # orig -> core
    loc_of = newpos // P_CORES                     # orig -> local pos (0..6249)
    grow_of = core_of * NPP + loc_of               # orig -> gather row

    # per-core slot lists. Edge (s,d): slot under (core_of[d], loc_of[d]).
    # Self loop appended for every node.
    s_all = np.concatenate([src, np.arange(N, dtype=np.int64)])
    d_all = np.concatenate([dst, np.arange(N, dtype=np.int64)])
    gs = grow_of[s_all]                            # gather row of src
    dcore = core_of[d_all]
    dloc = loc_of[d_all]
    is_lo = gs < SPLIT

    # order edges by (core, dloc, lo/hi) and compute ranks within group
    key = ((dcore * NP + dloc) * 2 + (~is_lo).astype(np.int64))
    order = np.argsort(key, kind="stable")
    ks = key[order]
    first = np.ones(len(ks), dtype=bool)
    first[1:] = ks[1:] != ks[:-1]
    starts = np.flatnonzero(first)
    run_id = np.cumsum(first) - 1
    rnk = np.arange(len(ks)) - starts[run_id]

    gso = gs[order]
    dco = dcore[order]
    dlo = dloc[order]
    loo = is_lo[order]

    # per (core, loc) lo/hi counts
    cnt_lo = np.zeros((P_CORES, NPP), dtype=np.int64)
    cnt_hi = np.zeros((P_CORES, NPP), dtype=np.int64)
    np.add.at(cnt_lo, (dcore[is_lo], dloc[is_lo]), 1)
    np.add.at(cnt_hi, (dcore[~is_lo], dloc[~is_lo]), 1)

    # per-group (4 tiles) capacities, max across cores
    R_lo = np.zeros(NG, dtype=np.int64)
    R_hi = np.zeros(NG, dtype=np.int64)
    for g in range(NG):
        l0, l1 = g * GSZ * 128, min((g + 1) * GSZ * 128, NPP)
        R_lo[g] = max(1, int(cnt_lo[:, l0:l1].max()))
        R_hi[g] = max(1, int(cnt_hi[:, l0:l1].max()))
    gt = [min(GSZ, NT - g * GSZ) for g in range(NG)]   # tiles in group

    # idx arrays (slot j inside group g half: j = (q*128+p)*R + r)
    n_lo = [int(gt[g] * 128 * R_lo[g]) for g in range(NG)]
    n_hi = [int(gt[g] * 128 * R_hi[g]) for g in range(NG)]
    off_lo = np.concatenate([[0], np.cumsum(n_lo)]).astype(np.int64)
    off_hi = np.concatenate([[0], np.cumsum(n_hi)]).astype(np.int64)
    tot_lo, tot_hi = int(off_lo[-1]), int(off_hi[-1])

    DUM_LO = NP            # core0 pad row 6250 (zero, < 32768)
    DUM_HI = 7 * NPP + NP  # core7 pad row (zero, >= 32768)
    idx_lo = np.full((P_CORES, tot_lo), DUM_LO, dtype=np.int64)
    idx_hi = np.full((P_CORES, tot_hi), DUM_HI - SPLIT, dtype=np.int64)

    g_of_loc = dlo // (GSZ * 128)
    qp = dlo - g_of_loc * (GSZ * 128)              # (q*128+p) within group
    sel = loo
    j = off_lo[g_of_loc[sel]] + qp[sel] * R_lo[g_of_loc[sel]] + rnk[sel]
    idx_lo[dco[sel], j] = gso[sel]
    sel = ~loo
    j = off_hi[g_of_loc[sel]] + qp[sel] * R_hi[g_of_loc[sel]] + rnk[sel]
    idx_hi[dco[sel], j] = gso[sel] - SPLIT

    def wrap16(a, tot):
        cols = tot // 16
        t = np.zeros((P_CORES, 128, cols), dtype=np.int16)
        v = a.astype(np.int16).reshape(P_CORES, cols, 16)
        for rs in range(0, 128, 16):
            t[:, rs:rs + 16, :] = v.transpose(0, 2, 1)
        return t

    assert tot_lo % 16 == 0 and tot_hi % 16 == 0
    idx_lo_t = wrap16(idx_lo, tot_lo)
    idx_hi_t = wrap16(idx_hi, tot_hi)

    # per-core dis factors in local-pos order
    dis_loc = np.zeros((P_CORES, NPP), dtype=np.float32)
    for c in range(P_CORES):
        n_ids = rank_of[c::P_CORES]                # local pos -> orig node
        dis_loc[c, :len(n_ids)] = dis[n_ids]
    disrow = np.broadcast_to(dis_loc[:, None, :], (P_CORES, 64, NPP)).copy()
    disrow2 = (disrow * disrow).copy()
    dinv = np.where(dis_loc > 0, 1.0 / np.maximum(dis_loc, 1e-9), 0.0)
    dinv = dinv.reshape(P_CORES, 1, NPP).astype(np.float32)

    return dict(dis=dis, rank_of=rank_of, grow_of=grow_of,
                R_lo=R_lo, R_hi=R_hi, gt=gt, n_lo=n_lo, n_hi=n_hi,
                off_lo=off_lo, off_hi=off_hi, tot_lo=tot_lo, tot_hi=tot_hi,
                idx_lo=idx_lo_t, idx_hi=idx_hi_t,
                disrow=disrow, disrow2=disrow2, dinv=dinv)


# ---------------------------------------------------------------------------
# Bass kernel
# ---------------------------------------------------------------------------

def _build_nc(prep):
    import concourse.bass as bass
    import concourse.bacc as bacc
    import concourse.tile as tile
    from concourse import mybir

    f32 = mybir.dt.float32
    bf16 = mybir.dt.bfloat16
    i16 = mybir.dt.int16
    AF = mybir.ActivationFunctionType
    ALU = mybir.AluOpType
    AX = mybir.AxisListType
    ds = bass.ds

    R_lo, R_hi, gt = prep["R_lo"], prep["R_hi"], prep["gt"]
    off_lo, off_hi = prep["off_lo"], prep["off_hi"]
    tot_lo, tot_hi = prep["tot_lo"], prep["tot_hi"]

    nc = bacc.Bacc(
        "TRN2", target_bir_lowering=False, debug=False,
        enable_asserts=False, num_devices=P_CORES,
    )

    xt_d = nc.dram_tensor("xt", [NROWS, 128], bf16, kind="ExternalInput")
    ixlo_d = nc.dram_tensor("ixlo", [128, tot_lo // 16], i16,
                            kind="ExternalInput")
    ixhi_d = nc.dram_tensor("ixhi", [128, tot_hi // 16], i16,
                            kind="ExternalInput")
    disr_d = nc.dram_tensor("disr", [64, NPP], f32, kind="ExternalInput")
    disr2_d = nc.dram_tensor("disr2", [64, NPP], bf16, kind="ExternalInput")
    dinv_d = nc.dram_tensor("dinv", [1, NPP], f32, kind="ExternalInput")
    w1_d = nc.dram_tensor("w1", [DIN, DH], f32, kind="ExternalInput")
    w2_d = nc.dram_tensor("w2b", [DH, DOUT], bf16, kind="ExternalInput")
    b1_d = nc.dram_tensor("b1r", [1, DH], f32, kind="ExternalInput")
    b2_d = nc.dram_tensor("b2c", [64, 1], f32, kind="ExternalInput")
    identb_d = nc.dram_tensor("identb", [128, 128], bf16, kind="ExternalInput")
    identf_d = nc.dram_tensor("identf", [64, 64], f32, kind="ExternalInput")
    out_d = nc.dram_tensor("outp", [NPP, DOUT], f32, kind="ExternalOutput")

    with tile.TileContext(nc) as tc:
        with (
            tc.tile_pool(name="const", bufs=1) as constp,
            tc.tile_pool(name="glo", bufs=2) as glop,
            tc.tile_pool(name="ghi", bufs=2) as ghip,
            tc.tile_pool(name="red", bufs=2) as redp,
            tc.tile_pool(name="wk", bufs=2) as work,
            tc.tile_pool(name="stg", bufs=2) as stgp,
            tc.tile_pool(name="px1", bufs=2, space="PSUM") as px1,
            tc.tile_pool(name="pp2", bufs=2, space="PSUM") as pp2,
            tc.tile_pool(name="ptr", bufs=2, space="PSUM") as ptr,
            tc.tile_pool(name="dram", bufs=1, space="DRAM") as dram,
        ):
            ixlo_sb = constp.tile([128, tot_lo // 16], i16)
            ixhi_sb = constp.tile([128, tot_hi // 16], i16)
            disr_sb = constp.tile([64, NPP], f32)
            disr2_sb = constp.tile([64, NPP], bf16)
            dinv_sb = constp.tile([1, NPP], f32)
            w1_sb = constp.tile([DIN, DH], f32)
            w2_sb = constp.tile([DH, DOUT], bf16)
            b1_sb = constp.tile([1, DH], f32)
            b2_sb = constp.tile([64, 1], f32)
            identb_sb = constp.tile([128, 128], bf16)
            identf_sb = constp.tile([64, 64], f32)
            for sb, dr in [(ixlo_sb, ixlo_d), (ixhi_sb, ixhi_d),
                           (disr_sb, disr_d), (disr2_sb, disr2_d),
                           (dinv_sb, dinv_d), (w1_sb, w1_d), (w2_sb, w2_d),
                           (b1_sb, b1_d), (b2_sb, b2_d),
                           (identb_sb, identb_d), (identf_sb, identf_d)]:
                nc.sync.dma_start(sb[:], dr[:])

            for _rep in range(int(os.environ.get("GCN_REPEAT", "1"))):
                part = dram.tile([NPP, 128], bf16, tag="part", bufs=2)
                table = dram.tile([NROWS, 128], bf16, addr_space="Shared",
                                  tag="table", bufs=2)

                for lidx in range(2):
                    tab = xt_d if lidx == 0 else table
                    for g in range(NG):
                        nt = gt[g]
                        nd = nt * 128            # dst columns in group
                        rl, rh = int(R_lo[g]), int(R_hi[g])
                        nlo, nhi = nd * rl, nd * rh
                        glo = glop.tile([128, nlo], bf16, tag="glo")
                        nc.gpsimd.dma_gather(
                            glo[:].rearrange("p (one n) -> p one n", one=1),
                            tab[0:SPLIT, :],
                            ixlo_sb[:, ds(int(off_lo[g]) // 16, nlo // 16)],
                            num_idxs=nlo, num_idxs_reg=nlo, elem_size=128,
                            transpose=True, single_packet=False)
                        ghi = ghip.tile([128, nhi], bf16, tag="ghi")
                        nc.gpsimd.dma_gather(
                            ghi[:].rearrange("p (one n) -> p one n", one=1),
                            tab[SPLIT:NROWS, :],
                            ixhi_sb[:, ds(int(off_hi[g]) // 16, nhi // 16)],
                            num_idxs=nhi, num_idxs_reg=nhi, elem_size=128,
                            transpose=True, single_packet=False)
                        r1 = redp.tile([64, nd], f32, tag="r1")
                        nc.vector.tensor_reduce(
                            r1[:],
                            glo[0:64, :].rearrange("p (d r) -> p d r", r=rl),
                            axis=AX.X, op=ALU.add)
                        r2 = redp.tile([64, nd], f32, tag="r2")
                        nc.vector.tensor_reduce(
                            r2[:],
                            ghi[0:64, :].rearrange("p (d r) -> p d r", r=rh),
                            axis=AX.X, op=ALU.add)
                        aggT = redp.tile([64, nd], f32, tag="aggT")
                        nc.vector.tensor_tensor(aggT[:], r1[:], r2[:],
                                                op=ALU.add)
                        col0 = g * GSZ * 128
                        if lidx == 0:
                            x1p = px1.tile([DH, nd], f32, tag="x1p")
                            nc.tensor.matmul(x1p[:], w1_sb[:], aggT[:],
                                             start=True, stop=False)
                            nc.tensor.matmul(
                                x1p[:], b1_sb[:],
                                dinv_sb[:, ds(col0, nd)],
                                start=False, stop=True)
                            x1sb = work.tile([DH, nd], bf16, tag="x1sb")
                            nc.scalar.activation(x1sb[:], x1p[:], AF.Lrelu,
                                                 bias=0.0, alpha=NEG_SLOPE)
                            p2p = pp2.tile([64, nd], f32, tag="p2p")
                            nc.tensor.matmul(p2p[:], w2_sb[:], x1sb[:],
                                             start=True, stop=True)
                            pts = work.tile([64, nd], bf16, tag="pts")
                            nc.vector.tensor_tensor(
                                pts[:], p2p[:], disr2_sb[:, ds(col0, nd)],
                                op=ALU.mult)
                            pT = ptr.tile([128, nt * 64], bf16, tag="pT")
                            for q in range(nt):
                                nc.tensor.transpose(
                                    pT[:, q * 64:(q + 1) * 64],
                                    pts[:, q * 128:(q + 1) * 128],
                                    identb_sb[0:64, 0:64])
                            stage = stgp.tile([128, nt * 64], bf16,
                                              tag="stage")
                            nc.scalar.activation(stage[:], pT[:], AF.Copy,
                                                 bias=0.0)
                            nc.sync.dma_start(
                                part[ds(col0, nd), 0:64].rearrange(
                                    "(q p) f -> p q f", p=128),
                                stage[:].rearrange("p (q f) -> p q f", f=64))
                        else:
                            aggs = work.tile([64, nd], f32, tag="aggs")
                            nc.vector.tensor_tensor(
                                aggs[:], aggT[:], disr_sb[:, ds(col0, nd)],
                                op=ALU.mult)
                            osbT = work.tile([64, nd], f32, tag="osbT")
                            nc.scalar.activation(osbT[:], aggs[:], AF.Lrelu,
                                                 bias=b2_sb[:, 0:1],
                                                 alpha=NEG_SLOPE)
                            oT = ptr.tile([128, nt * 64], f32, tag="oT")
                            for q in range(nt):
                                nc.tensor.transpose(
                                    oT[:, q * 64:(q + 1) * 64],
                                    osbT[:, q * 128:(q + 1) * 128],
                                    identf_sb[:])
                            ost = stgp.tile([128, nt * 64], f32, tag="ost")
                            nc.scalar.activation(ost[:], oT[:], AF.Copy,
                                                 bias=0.0)
                            nc.sync.dma_start(
                                out_d[ds(col0, nd), :].rearrange(
                                    "(q p) f -> p q f", p=128),
                                ost[:].rearrange("p (q f) -> p q f", f=64))
                    if lidx == 0:
                        if os.environ.get("GCN_NOAG", "0") == "1":
                            nc.sync.dma_start(table[0:NPP, :], part[:, :])
                        else:
                            nc.gpsimd.collective_compute(
                                "AllGather", mybir.AluOpType.bypass,
                                replica_groups=[list(range(P_CORES))],
                                ins=[part.opt()], outs=[table.opt()],
                            )

    nc.compile()
    return nc


def _make_in_maps(inputs, W1, b1, W2, b2, prep):
    import ml_dtypes
    dis = prep["dis"]
    rank_of = prep["rank_of"]
    x32 = np.asarray(inputs, np.float32) * dis[:, None]   # dis_s * x_s
    xt = np.zeros((NROWS, 128), dtype=np.float32)
    # gather row (k%8)*NPP + k//8 holds rank-k node
    k = np.arange(N)
    rows = (k % P_CORES) * NPP + (k // P_CORES)
    xt[rows, :DIN] = x32[rank_of]
    xt = xt.astype(ml_dtypes.bfloat16)
    ident = np.eye(128, dtype=np.float32)
    in_maps = []
    for c in range(P_CORES):
        in_maps.append({
            "xt": xt,
            "ixlo": prep["idx_lo"][c],
            "ixhi": prep["idx_hi"][c],
            "disr": prep["disrow"][c],
            "disr2": prep["disrow2"][c].astype(ml_dtypes.bfloat16),
            "dinv": prep["dinv"][c],
            "w1": np.asarray(W1, np.float32),
            "w2b": np.asarray(W2, np.float32).astype(ml_dtypes.bfloat16),
            "b1r": np.asarray(b1, np.float32).reshape(1, DH),
            "b2c": np.asarray(b2, np.float32).reshape(64, 1),
            "identb": ident.astype(ml_dtypes.bfloat16),
            "identf": ident[:64, :64].copy(),
        })
    return in_maps


_CACHE = {}


def kernel(inputs, edge_index, W1, b1, W2, b2, _trace=False, _results_box=None):
    from concourse.bass_utils import run_bass_kernel_spmd

    edge_index = np.asarray(edge_index)
    key = hashlib.sha1(edge_index.tobytes()).hexdigest()
    key += ":r%s:n%s" % (os.environ.get("GCN_REPEAT", "1"),
                         os.environ.get("GCN_NOAG", "0"))
    if key not in _CACHE:
        prep = _prep(edge_index)
        nc = _build_nc(prep)
        _CACHE[key] = (prep, nc)
    prep, nc = _CACHE[key]
    in_maps = _make_in_maps(inputs, W1, b1, W2, b2, prep)
    res = run_bass_kernel_spmd(
        nc, in_maps, core_ids=list(range(P_CORES)), trace=_trace,
    )
    if _results_box is not None:
        _results_box.append(res)
    # core c local pos l holds rank (l*8 + c) -> orig node rank_of[...]
    outp = np.empty((N, DOUT), dtype=np.float32)
    rank_of = prep["rank_of"]
    for c in range(P_CORES):
        o = res.results[c]["outp"][:NP]              # local pos order
        ranks = np.arange(NP) * P_CORES + c
        outp[rank_of[ranks]] = o
    return outp


# revision 7
# speedup vs baseline: 4.7606x; 1.1971x over previous
"""2-layer GCN (PyG GCNConv x2 + leaky_relu) on 8 Trainium2 NeuronCores.

v3 strategy (gather + segment-reduce; tuned for the per-instruction-cost
regime of this stack):
  - Nodes are ranked by degree and dealt round-robin across the 8 cores
    (rank k -> core k%8, local slot k//8), so every core's tile t holds
    nodes of nearly identical degree. Per-tile slot capacities R_t are
    then tight and identical across cores (SPMD-uniform program).
  - Per dst tile (128 nodes): one dma_gather per int16-range half pulls
    all in-edge messages (and the self-loop row) into SBUF feature-major
    [128 feat, d*R + r]; a single DVE tensor_reduce per half does the
    segment sum; add merges halves. Tiles are processed in groups of 4
    (512 dst columns per instruction) to amortize the ~35us/instruction
    floor.
  - Tables are [rows, 128] bf16 (256B rows as dma_gather requires; cols
    64..127 are junk and never read). idxs are int16, so the table is
    split at row 32768 into lo/hi gathers; dummy slots point at
    guaranteed-zero pad rows.
  - Normalization: table rows pre-scaled by dis[src] (host for layer 1;
    the layer-1 tail emits dis^2 * lrelu(aggW1 + dinv b1) W2 so the
    AllGather'd table is already source-scaled). dis[dst] applied via
    precomputed [64, 6272] row-replicated factors (DVE multiply).
  - Layer tail runs feature-major: W1 matmul + rank-1 b1*dinv into PSUM,
    ACT lrelu, W2 matmul, DVE dis^2 scale, 4x PE transpose, ACT copy,
    one strided store per group. One AllGather builds the layer-2 table.

Self-contained: hardcodes shapes; compiles on first call keyed by edge hash.
"""

import os
import hashlib
import sys

import numpy as np

sys.path.insert(0, "/opt/trn_rl_repo")

# ---- problem constants ----
N, E = 50000, 800000
DIN, DH, DOUT = 64, 128, 64
P_CORES = 8
NP = N // P_CORES            # 6250 real nodes per core
NT = 49                      # dst tiles per core
NPP = NT * 128               # 6272 padded rows per core
NROWS = P_CORES * NPP        # 50176 table rows
SPLIT = 32768                # int16 gather index limit
GSZ = 4                      # tiles per group
NG = (NT + GSZ - 1) // GSZ   # 13 groups (12x4 + 1x1)
NEG_SLOPE = 0.01


def _prep(edge_index: np.ndarray):
    src = np.asarray(edge_index[0], dtype=np.int64)
    dst = np.asarray(edge_index[1], dtype=np.int64)

    deg = (np.bincount(dst, minlength=N) + 1).astype(np.float32)
    dis = (1.0 / np.sqrt(deg)).astype(np.float32)

    # degree-descending rank -> (core, local pos)
    rank_of = np.argsort(-deg, kind="stable")      # rank -> orig node
    newpos = np.empty(N, dtype=np.int64)           # orig -> rank
    newpos[rank_of] = np.arange(N)
    core_of = newpos % P_CORES                     # orig -> core
    loc_of = newpos // P_CORES                     # orig -> local pos (0..6249)
    grow_of = core_of * NPP + loc_of               # orig -> gather row

    # per-core slot lists. Edge (s,d): slot under (core_of[d], loc_of[d]).
    # Self loop appended for every node.
    s_all = np.concatenate([src, np.arange(N, dtype=np.int64)])
    d_all = np.concatenate([dst, np.arange(N, dtype=np.int64)])
    gs = grow_of[s_all]                            # gather row of src
    dcore = core_of[d_all]
    dloc = loc_of[d_all]
    is_lo = gs < SPLIT

    # order edges by (core, dloc, lo/hi) and compute ranks within group
    key = ((dcore * NP + dloc) * 2 + (~is_lo).astype(np.int64))
    order = np.argsort(key, kind="stable")
    ks = key[order]
    first = np.ones(len(ks), dtype=bool)
    first[1:] = ks[1:] != ks[:-1]
    starts = np.flatnonzero(first)
    run_id = np.cumsum(first) - 1
    rnk = np.arange(len(ks)) - starts[run_id]

    gso = gs[order]
    dco = dcore[order]
    dlo = dloc[order]
    loo = is_lo[order]

    # per (core, loc) lo/hi counts
    cnt_lo = np.zeros((P_CORES, NPP), dtype=np.int64)
    cnt_hi = np.zeros((P_CORES, NPP), dtype=np.int64)
    np.add.at(cnt_lo, (dcore[is_lo], dloc[is_lo]), 1)
    np.add.at(cnt_hi, (dcore[~is_lo], dloc[~is_lo]), 1)

    # per-group (4 tiles) capacities, max across cores
    R_lo = np.zeros(NG, dtype=np.int64)
    R_hi = np.zeros(NG, dtype=np.int64)
    for g in range(NG):
        l0, l1 = g * GSZ * 128, min((g + 1) * GSZ * 128, NPP)
        R_lo[g] = max(1, int(cnt_lo[:, l0:l1].max()))
        R_hi[g] = max(1, int(cnt_hi[:, l0:l1].max()))
    gt = [min(GSZ, NT - g * GSZ) for g in range(NG)]   # tiles in group

    # idx arrays (slot j inside group g half: j = (q*128+p)*R + r)
    n_lo = [int(gt[g] * 128 * R_lo[g]) for g in range(NG)]
    n_hi = [int(gt[g] * 128 * R_hi[g]) for g in range(NG)]
    off_lo = np.concatenate([[0], np.cumsum(n_lo)]).astype(np.int64)
    off_hi = np.concatenate([[0], np.cumsum(n_hi)]).astype(np.int64)
    tot_lo, tot_hi = int(off_lo[-1]), int(off_hi[-1])

    DUM_LO = NP            # core0 pad row 6250 (zero, < 32768)
    DUM_HI = 7 * NPP + NP  # core7 pad row (zero, >= 32768)
    idx_lo = np.full((P_CORES, tot_lo), DUM_LO, dtype=np.int64)
    idx_hi = np.full((P_CORES, tot_hi), DUM_HI - SPLIT, dtype=np.int64)

    g_of_loc = dlo // (GSZ * 128)
    qp = dlo - g_of_loc * (GSZ * 128)              # (q*128+p) within group
    sel = loo
    j = off_lo[g_of_loc[sel]] + qp[sel] * R_lo[g_of_loc[sel]] + rnk[sel]
    idx_lo[dco[sel], j] = gso[sel]
    sel = ~loo
    j = off_hi[g_of_loc[sel]] + qp[sel] * R_hi[g_of_loc[sel]] + rnk[sel]
    idx_hi[dco[sel], j] = gso[sel] - SPLIT

    def wrap16(a, tot):
        cols = tot // 16
        t = np.zeros((P_CORES, 128, cols), dtype=np.int16)
        v = a.astype(np.int16).reshape(P_CORES, cols, 16)
        for rs in range(0, 128, 16):
            t[:, rs:rs + 16, :] = v.transpose(0, 2, 1)
        return t

    assert tot_lo % 16 == 0 and tot_hi % 16 == 0
    idx_lo_t = wrap16(idx_lo, tot_lo)
    idx_hi_t = wrap16(idx_hi, tot_hi)

    # per-core dis factors in local-pos order
    dis_loc = np.zeros((P_CORES, NPP), dtype=np.float32)
    for c in range(P_CORES):
        n_ids = rank_of[c::P_CORES]                # local pos -> orig node
        dis_loc[c, :len(n_ids)] = dis[n_ids]
    disrow = np.broadcast_to(dis_loc[:, None, :], (P_CORES, 64, NPP)).copy()
    disrow2 = (disrow * disrow).copy()
    dinv = np.where(dis_loc > 0, 1.0 / np.maximum(dis_loc, 1e-9), 0.0)
    dinv = dinv.reshape(P_CORES, 1, NPP).astype(np.float32)

    return dict(dis=dis, rank_of=rank_of, grow_of=grow_of,
                R_lo=R_lo, R_hi=R_hi, gt=gt, n_lo=n_lo, n_hi=n_hi,
                off_lo=off_lo, off_hi=off_hi, tot_lo=tot_lo, tot_hi=tot_hi,
                idx_lo=idx_lo_t, idx_hi=idx_hi_t,
                disrow=disrow, disrow2=disrow2, dinv=dinv)


# ---------------------------------------------------------------------------
# Bass kernel
# ---------------------------------------------------------------------------

def _build_nc(prep):
    import concourse.bass as bass
    import concourse.bacc as bacc
    import concourse.tile as tile
    from concourse import mybir

    f32 = mybir.dt.float32
    bf16 = mybir.dt.bfloat16
    i16 = mybir.dt.int16
    AF = mybir.ActivationFunctionType
    ALU = mybir.AluOpType
    AX = mybir.AxisListType
    ds = bass.ds

    R_lo, R_hi, gt = prep["R_lo"], prep["R_hi"], prep["gt"]
    off_lo, off_hi = prep["off_lo"], prep["off_hi"]
    tot_lo, tot_hi = prep["tot_lo"], prep["tot_hi"]

    nc = bacc.Bacc(
        "TRN2", target_bir_lowering=False, debug=False,
        enable_asserts=False, num_devices=P_CORES,
    )

    xt_d = nc.dram_tensor("xt", [NROWS, 128], bf16, kind="ExternalInput")
    ixlo_d = nc.dram_tensor("ixlo", [128, tot_lo // 16], i16,
                            kind="ExternalInput")
    ixhi_d = nc.dram_tensor("ixhi", [128, tot_hi // 16], i16,
                            kind="ExternalInput")
    disr_d = nc.dram_tensor("disr", [64, NPP], f32, kind="ExternalInput")
    disr2_d = nc.dram_tensor("disr2", [64, NPP], bf16, kind="ExternalInput")
    dinv_d = nc.dram_tensor("dinv", [1, NPP], f32, kind="ExternalInput")
    w1_d = nc.dram_tensor("w1", [DIN, DH], f32, kind="ExternalInput")
    w2_d = nc.dram_tensor("w2b", [DH, DOUT], bf16, kind="ExternalInput")
    b1_d = nc.dram_tensor("b1r", [1, DH], f32, kind="ExternalInput")
    b2_d = nc.dram_tensor("b2c", [64, 1], f32, kind="ExternalInput")
    identb_d = nc.dram_tensor("identb", [128, 128], bf16, kind="ExternalInput")
    identf_d = nc.dram_tensor("identf", [64, 64], f32, kind="ExternalInput")
    out_d = nc.dram_tensor("outp", [NPP, DOUT], f32, kind="ExternalOutput")

    with tile.TileContext(nc) as tc:
        with (
            tc.tile_pool(name="const", bufs=1) as constp,
            tc.tile_pool(name="glo", bufs=2) as glop,
            tc.tile_pool(name="ghi", bufs=2) as ghip,
            tc.tile_pool(name="red", bufs=2) as redp,
            tc.tile_pool(name="wk", bufs=2) as work,
            tc.tile_pool(name="stg", bufs=2) as stgp,
            tc.tile_pool(name="px1", bufs=2, space="PSUM") as px1,
            tc.tile_pool(name="pp2", bufs=2, space="PSUM") as pp2,
            tc.tile_pool(name="ptr", bufs=2, space="PSUM") as ptr,
            tc.tile_pool(name="dram", bufs=1, space="DRAM") as dram,
        ):
            ixlo_sb = constp.tile([128, tot_lo // 16], i16)
            ixhi_sb = constp.tile([128, tot_hi // 16], i16)
            disr_sb = constp.tile([64, NPP], f32)
            disr2_sb = constp.tile([64, NPP], bf16)
            dinv_sb = constp.tile([1, NPP], f32)
            w1_sb = constp.tile([DIN, DH], f32)
            w2_sb = constp.tile([DH, DOUT], bf16)
            b1_sb = constp.tile([1, DH], f32)
            b2_sb = constp.tile([64, 1], f32)
            identb_sb = constp.tile([128, 128], bf16)
            identf_sb = constp.tile([64, 64], f32)
            for sb, dr in [(ixlo_sb, ixlo_d), (ixhi_sb, ixhi_d),
                           (disr_sb, disr_d), (disr2_sb, disr2_d),
                           (dinv_sb, dinv_d), (w1_sb, w1_d), (w2_sb, w2_d),
                           (b1_sb, b1_d), (b2_sb, b2_d),
                           (identb_sb, identb_d), (identf_sb, identf_d)]:
                nc.sync.dma_start(sb[:], dr[:])

            for _rep in range(int(os.environ.get("GCN_REPEAT", "1"))):
                part = dram.tile([NPP, 64], bf16, tag="part", bufs=2)
                tab64 = dram.tile([NROWS, 64], bf16, addr_space="Shared",
                                  tag="tab64", bufs=2)
                table = dram.tile([NROWS, 128], bf16, tag="table", bufs=2)

                abl = os.environ.get("GCN_ABL", "")
                for lidx in range(2):
                    tab = xt_d if lidx == 0 else table
                    for g in range(NG):
                        nt = gt[g]
                        nd = nt * 128            # dst columns in group
                        rl, rh = int(R_lo[g]), int(R_hi[g])
                        nlo, nhi = nd * rl, nd * rh
                        glo = glop.tile([128, nlo], bf16, tag="glo")
                        if abl != "nog":
                         nc.gpsimd.dma_gather(
                            glo[:].rearrange("p (one n) -> p one n", one=1),
                            tab[0:SPLIT, :],
                            ixlo_sb[:, ds(int(off_lo[g]) // 16, nlo // 16)],
                            num_idxs=nlo, num_idxs_reg=nlo, elem_size=128,
                            transpose=True, single_packet=False)
                        ghi = ghip.tile([128, nhi], bf16, tag="ghi")
                        if abl != "nog":
                         nc.gpsimd.dma_gather(
                            ghi[:].rearrange("p (one n) -> p one n", one=1),
                            tab[SPLIT:NROWS, :],
                            ixhi_sb[:, ds(int(off_hi[g]) // 16, nhi // 16)],
                            num_idxs=nhi, num_idxs_reg=nhi, elem_size=128,
                            transpose=True, single_packet=False)
                        r1 = redp.tile([64, nd], f32, tag="r1")
                        nc.vector.tensor_reduce(
                            r1[:],
                            glo[0:64, :].rearrange("p (d r) -> p d r", r=rl),
                            axis=AX.X, op=ALU.add)
                        r2 = redp.tile([64, nd], f32, tag="r2")
                        nc.vector.tensor_reduce(
                            r2[:],
                            ghi[0:64, :].rearrange("p (d r) -> p d r", r=rh),
                            axis=AX.X, op=ALU.add)
                        aggT = redp.tile([64, nd], f32, tag="aggT")
                        nc.vector.tensor_tensor(aggT[:], r1[:], r2[:],
                                                op=ALU.add)
                        col0 = g * GSZ * 128
                        if abl == "gonly":
                            if lidx == 1:
                                ost = stgp.tile([64, 64], f32, tag="ostg")
                                nc.scalar.activation(
                                    ost[:], aggT[0:64, 0:64], AF.Copy,
                                    bias=0.0)
                                nc.sync.dma_start(
                                    out_d[ds(col0, 64), :], ost[:])
                            continue
                        if lidx == 0:
                            x1p = px1.tile([DH, nd], f32, tag="x1p")
                            nc.tensor.matmul(x1p[:], w1_sb[:], aggT[:],
                                             start=True, stop=False)
                            nc.tensor.matmul(
                                x1p[:], b1_sb[:],
                                dinv_sb[:, ds(col0, nd)],
                                start=False, stop=True)
                            x1sb = work.tile([DH, nd], bf16, tag="x1sb")
                            nc.scalar.activation(x1sb[:], x1p[:], AF.Lrelu,
                                                 bias=0.0, alpha=NEG_SLOPE)
                            p2p = pp2.tile([64, nd], f32, tag="p2p")
                            nc.tensor.matmul(p2p[:], w2_sb[:], x1sb[:],
                                             start=True, stop=True)
                            pts = work.tile([64, nd], bf16, tag="pts")
                            nc.vector.tensor_tensor(
                                pts[:], p2p[:], disr2_sb[:, ds(col0, nd)],
                                op=ALU.mult)
                            pT = ptr.tile([128, nt * 64], bf16, tag="pT")
                            for q in range(nt):
                                nc.tensor.transpose(
                                    pT[:, q * 64:(q + 1) * 64],
                                    pts[:, q * 128:(q + 1) * 128],
                                    identb_sb[0:64, 0:64])
                            stage = stgp.tile([128, nt * 64], bf16,
                                              tag="stage")
                            nc.scalar.activation(stage[:], pT[:], AF.Copy,
                                                 bias=0.0)
                            nc.sync.dma_start(
                                part[ds(col0, nd), :].rearrange(
                                    "(q p) f -> p q f", p=128),
                                stage[:].rearrange("p (q f) -> p q f", f=64))
                        else:
                            aggs = work.tile([64, nd], f32, tag="aggs")
                            nc.vector.tensor_tensor(
                                aggs[:], aggT[:], disr_sb[:, ds(col0, nd)],
                                op=ALU.mult)
                            osbT = work.tile([64, nd], f32, tag="osbT")
                            nc.scalar.activation(osbT[:], aggs[:], AF.Lrelu,
                                                 bias=b2_sb[:, 0:1],
                                                 alpha=NEG_SLOPE)
                            oT = ptr.tile([128, nt * 64], f32, tag="oT")
                            for q in range(nt):
                                nc.tensor.transpose(
                                    oT[:, q * 64:(q + 1) * 64],
                                    osbT[:, q * 128:(q + 1) * 128],
                                    identf_sb[:])
                            ost = stgp.tile([128, nt * 64], f32, tag="ost")
                            nc.scalar.activation(ost[:], oT[:], AF.Copy,
                                                 bias=0.0)
                            nc.sync.dma_start(
                                out_d[ds(col0, nd), :].rearrange(
                                    "(q p) f -> p q f", p=128),
                                ost[:].rearrange("p (q f) -> p q f", f=64))
                    if lidx == 0:
                        if os.environ.get("GCN_NOAG", "0") == "1":
                            nc.sync.dma_start(tab64[0:NPP, :], part[:, :])
                        else:
                            nc.gpsimd.collective_compute(
                                "AllGather", mybir.AluOpType.bypass,
                                replica_groups=[list(range(P_CORES))],
                                ins=[part.opt()], outs=[tab64.opt()],
                            )
                        nc.sync.dma_start(table[:, 0:64], tab64[:, :])

    nc.compile()
    return nc


def _make_in_maps(inputs, W1, b1, W2, b2, prep):
    import ml_dtypes
    dis = prep["dis"]
    rank_of = prep["rank_of"]
    x32 = np.asarray(inputs, np.float32) * dis[:, None]   # dis_s * x_s
    xt = np.zeros((NROWS, 128), dtype=np.float32)
    # gather row (k%8)*NPP + k//8 holds rank-k node
    k = np.arange(N)
    rows = (k % P_CORES) * NPP + (k // P_CORES)
    xt[rows, :DIN] = x32[rank_of]
    xt = xt.astype(ml_dtypes.bfloat16)
    ident = np.eye(128, dtype=np.float32)
    in_maps = []
    for c in range(P_CORES):
        in_maps.append({
            "xt": xt,
            "ixlo": prep["idx_lo"][c],
            "ixhi": prep["idx_hi"][c],
            "disr": prep["disrow"][c],
            "disr2": prep["disrow2"][c].astype(ml_dtypes.bfloat16),
            "dinv": prep["dinv"][c],
            "w1": np.asarray(W1, np.float32),
            "w2b": np.asarray(W2, np.float32).astype(ml_dtypes.bfloat16),
            "b1r": np.asarray(b1, np.float32).reshape(1, DH),
            "b2c": np.asarray(b2, np.float32).reshape(64, 1),
            "identb": ident.astype(ml_dtypes.bfloat16),
            "identf": ident[:64, :64].copy(),
        })
    return in_maps


_CACHE = {}


def kernel(inputs, edge_index, W1, b1, W2, b2, _trace=False, _results_box=None):
    from concourse.bass_utils import run_bass_kernel_spmd

    edge_index = np.asarray(edge_index)
    key = hashlib.sha1(edge_index.tobytes()).hexdigest()
    key += ":r%s:n%s:a%s" % (os.environ.get("GCN_REPEAT", "1"),
                             os.environ.get("GCN_NOAG", "0"),
                             os.environ.get("GCN_ABL", ""))
    if key not in _CACHE:
        prep = _prep(edge_index)
        nc = _build_nc(prep)
        _CACHE[key] = (prep, nc)
    prep, nc = _CACHE[key]
    in_maps = _make_in_maps(inputs, W1, b1, W2, b2, prep)
    res = run_bass_kernel_spmd(
        nc, in_maps, core_ids=list(range(P_CORES)), trace=_trace,
    )
    if _results_box is not None:
        _results_box.append(res)
    # core c local pos l holds rank (l*8 + c) -> orig node rank_of[...]
    outp = np.empty((N, DOUT), dtype=np.float32)
    rank_of = prep["rank_of"]
    for c in range(P_CORES):
        o = res.results[c]["outp"][:NP]              # local pos order
        ranks = np.arange(NP) * P_CORES + c
        outp[rank_of[ranks]] = o
    return outp


# revision 9
# speedup vs baseline: 6.9841x; 1.4671x over previous
"""2-layer GCN (PyG GCNConv x2 + leaky_relu) on 8 Trainium2 NeuronCores.

v4 strategy (pair-packed gather + segment-reduce):
  - Nodes ranked by degree, dealt round-robin across cores (rank k ->
    core k%8, local pos k//8): every core's tile t holds nodes of nearly
    identical degree, so slot capacities are tight and SPMD-uniform.
  - The node table is pair-packed: DRAM row m (256B) holds features of
    gather-rows 2m and 2m+1 (64 bf16 each). dma_gather with idx=row//2
    (always < 25088, int16-safe) lands pairs feature-major: partitions
    0..63 = even row, 64..127 = odd row. Per dst, in-edges (+ self loop)
    are split by src-row parity into even/odd slot grids; one gather per
    half per tile-group, one DVE tensor_reduce per half writing disjoint
    partition halves of a single [128, nd] accumulator.
  - The even/odd halves are merged by the tail matmul itself: W1 is
    duplicated across partitions 0..63/64..127 so the K=128 contraction
    sums both halves (layer 2 uses a stacked-identity matmul instead).
  - Tail per group (<=4 tiles, 512 dst columns/instruction): W1 matmul +
    rank-1 b1*dinv into PSUM, ACT lrelu, W2 matmul, DVE dis^2 scale,
    per-tile PE transpose, ACT copy, one strided store. Layer-2 table is
    built by a 64-col AllGather of the per-core part buffers (the pair
    view is just an AP reshape - no expand pass).
  - Group sizes adapt to per-tile slot caps so the worst gather buffers
    stay small enough for bufs=3 buffering (deep DMA pipeline).

Self-contained: hardcodes shapes; compiles on first call keyed by edge hash.
"""

import os
import hashlib
import sys

import numpy as np

sys.path.insert(0, "/opt/trn_rl_repo")

# ---- problem constants ----
N, E = 50000, 800000
DIN, DH, DOUT = 64, 128, 64
P_CORES = 8
NP = N // P_CORES            # 6250 real nodes per core
NT = 49                      # dst tiles per core
NPP = NT * 128               # 6272 padded rows per core
NROWS = P_CORES * NPP        # 50176 table rows
NPAIR = NROWS // 2           # 25088 pair rows
GSZ = 4                      # max tiles per group
CAP_SLOTS = 7168             # max slots per group-half buffer (14KB/part)
NEG_SLOPE = 0.01


def _prep(edge_index: np.ndarray):
    src = np.asarray(edge_index[0], dtype=np.int64)
    dst = np.asarray(edge_index[1], dtype=np.int64)

    deg = (np.bincount(dst, minlength=N) + 1).astype(np.float32)
    dis = (1.0 / np.sqrt(deg)).astype(np.float32)

    # degree-descending rank -> (core, local pos)
    rank_of = np.argsort(-deg, kind="stable")      # rank -> orig node
    newpos = np.empty(N, dtype=np.int64)           # orig -> rank
    newpos[rank_of] = np.arange(N)
    core_of = newpos % P_CORES
    loc_of = newpos // P_CORES
    grow_of = core_of * NPP + loc_of               # orig -> gather row

    # edge slot lists (self loop appended for every node)
    s_all = np.concatenate([src, np.arange(N, dtype=np.int64)])
    d_all = np.concatenate([dst, np.arange(N, dtype=np.int64)])
    gs = grow_of[s_all]
    dcore = core_of[d_all]
    dloc = loc_of[d_all]
    is_ev = (gs % 2) == 0

    key = ((dcore * NPP + dloc) * 2 + (~is_ev).astype(np.int64))
    order = np.argsort(key, kind="stable")
    ks = key[order]
    first = np.ones(len(ks), dtype=bool)
    first[1:] = ks[1:] != ks[:-1]
    starts = np.flatnonzero(first)
    run_id = np.cumsum(first) - 1
    rnk = np.arange(len(ks)) - starts[run_id]

    gso = gs[order]
    dco = dcore[order]
    dlo = dloc[order]
    evo = is_ev[order]

    cnt_ev = np.zeros((P_CORES, NPP), dtype=np.int64)
    cnt_od = np.zeros((P_CORES, NPP), dtype=np.int64)
    np.add.at(cnt_ev, (dcore[is_ev], dloc[is_ev]), 1)
    np.add.at(cnt_od, (dcore[~is_ev], dloc[~is_ev]), 1)

    # per-tile capacities (max across cores)
    Rt_ev = np.maximum(1, cnt_ev.reshape(P_CORES, NT, 128).max(axis=(0, 2)))
    Rt_od = np.maximum(1, cnt_od.reshape(P_CORES, NT, 128).max(axis=(0, 2)))

    # adaptive grouping: <= GSZ tiles and <= CAP_SLOTS per half
    groups = []                                    # list of (t0, nt)
    t = 0
    while t < NT:
        nt = 1
        while (t + nt < NT and nt < GSZ
               and (nt + 1) * 128 * max(Rt_ev[t:t + nt + 1]) <= CAP_SLOTS
               and (nt + 1) * 128 * max(Rt_od[t:t + nt + 1]) <= CAP_SLOTS):
            nt += 1
        groups.append((t, nt))
        t += nt
    R_ev = np.array([max(Rt_ev[t0:t0 + nt]) for t0, nt in groups])
    R_od = np.array([max(Rt_od[t0:t0 + nt]) for t0, nt in groups])

    n_ev = [int(nt * 128 * R_ev[g]) for g, (t0, nt) in enumerate(groups)]
    n_od = [int(nt * 128 * R_od[g]) for g, (t0, nt) in enumerate(groups)]
    off_ev = np.concatenate([[0], np.cumsum(n_ev)]).astype(np.int64)
    off_od = np.concatenate([[0], np.cumsum(n_od)]).astype(np.int64)
    tot_ev, tot_od = int(off_ev[-1]), int(off_od[-1])

    DUMMY = NP // 2        # pair row 3125 = rows (6250, 6251), core0 pads
    idx_ev = np.full((P_CORES, tot_ev), DUMMY, dtype=np.int64)
    idx_od = np.full((P_CORES, tot_od), DUMMY, dtype=np.int64)

    # map each loc to its group and position within the group
    g_of_tile = np.empty(NT, dtype=np.int64)
    qoff_of_tile = np.empty(NT, dtype=np.int64)    # (q*128) offset in group
    for g, (t0, nt) in enumerate(groups):
        g_of_tile[t0:t0 + nt] = g
        qoff_of_tile[t0:t0 + nt] = np.arange(nt) * 128
    tile_of = dlo // 128
    gg = g_of_tile[tile_of]
    qp = qoff_of_tile[tile_of] + (dlo % 128)

    sel = evo
    j = off_ev[gg[sel]] + qp[sel] * R_ev[gg[sel]] + rnk[sel]
    idx_ev[dco[sel], j] = gso[sel] // 2
    sel = ~evo
    j = off_od[gg[sel]] + qp[sel] * R_od[gg[sel]] + rnk[sel]
    idx_od[dco[sel], j] = gso[sel] // 2

    def wrap16(a, tot):
        cols = tot // 16
        t = np.zeros((P_CORES, 128, cols), dtype=np.int16)
        v = a.astype(np.int16).reshape(P_CORES, cols, 16)
        for rs in range(0, 128, 16):
            t[:, rs:rs + 16, :] = v.transpose(0, 2, 1)
        return t

    assert tot_ev % 16 == 0 and tot_od % 16 == 0
    idx_ev_t = wrap16(idx_ev, tot_ev)
    idx_od_t = wrap16(idx_od, tot_od)

    dis_loc = np.zeros((P_CORES, NPP), dtype=np.float32)
    for c in range(P_CORES):
        n_ids = rank_of[c::P_CORES]
        dis_loc[c, :len(n_ids)] = dis[n_ids]
    disrow = np.broadcast_to(dis_loc[:, None, :], (P_CORES, 64, NPP)).copy()
    disrow2 = (disrow * disrow).copy()
    dinv = np.where(dis_loc > 0, 1.0 / np.maximum(dis_loc, 1e-9), 0.0)
    dinv = dinv.reshape(P_CORES, 1, NPP).astype(np.float32)

    return dict(dis=dis, rank_of=rank_of,
                groups=groups, R_ev=R_ev, R_od=R_od,
                off_ev=off_ev, off_od=off_od, tot_ev=tot_ev, tot_od=tot_od,
                idx_ev=idx_ev_t, idx_od=idx_od_t,
                disrow=disrow, disrow2=disrow2, dinv=dinv)


# ---------------------------------------------------------------------------
# Bass kernel
# ---------------------------------------------------------------------------

def _build_nc(prep):
    import concourse.bass as bass
    import concourse.bacc as bacc
    import concourse.tile as tile
    from concourse import mybir

    f32 = mybir.dt.float32
    bf16 = mybir.dt.bfloat16
    i16 = mybir.dt.int16
    AF = mybir.ActivationFunctionType
    ALU = mybir.AluOpType
    AX = mybir.AxisListType
    ds = bass.ds

    groups = prep["groups"]
    R_ev, R_od = prep["R_ev"], prep["R_od"]
    off_ev, off_od = prep["off_ev"], prep["off_od"]
    tot_ev, tot_od = prep["tot_ev"], prep["tot_od"]

    nc = bacc.Bacc(
        "TRN2", target_bir_lowering=False, debug=False,
        enable_asserts=False, num_devices=P_CORES,
    )

    xt_d = nc.dram_tensor("xt", [NPAIR, 128], bf16, kind="ExternalInput")
    ixev_d = nc.dram_tensor("ixev", [128, tot_ev // 16], i16,
                            kind="ExternalInput")
    ixod_d = nc.dram_tensor("ixod", [128, tot_od // 16], i16,
                            kind="ExternalInput")
    disr_d = nc.dram_tensor("disr", [64, NPP], f32, kind="ExternalInput")
    disr2_d = nc.dram_tensor("disr2", [64, NPP], bf16, kind="ExternalInput")
    dinv_d = nc.dram_tensor("dinv", [1, NPP], f32, kind="ExternalInput")
    w1_d = nc.dram_tensor("w1d", [DH, DH], f32, kind="ExternalInput")
    w2_d = nc.dram_tensor("w2b", [DH, DOUT], bf16, kind="ExternalInput")
    b1_d = nc.dram_tensor("b1r", [1, DH], f32, kind="ExternalInput")
    b2_d = nc.dram_tensor("b2c", [64, 1], f32, kind="ExternalInput")
    identb_d = nc.dram_tensor("identb", [64, 64], bf16, kind="ExternalInput")
    identf_d = nc.dram_tensor("identf", [64, 64], f32, kind="ExternalInput")
    ident2_d = nc.dram_tensor("ident2", [DH, 64], f32, kind="ExternalInput")
    out_d = nc.dram_tensor("outp", [NPP, DOUT], f32, kind="ExternalOutput")

    with tile.TileContext(nc) as tc:
        with (
            tc.tile_pool(name="const", bufs=1) as constp,
            tc.tile_pool(name="gev", bufs=3) as gevp,
            tc.tile_pool(name="god", bufs=3) as godp,
            tc.tile_pool(name="red", bufs=3) as redp,
            tc.tile_pool(name="wk", bufs=2) as work,
            tc.tile_pool(name="stg", bufs=2) as stgp,
            tc.tile_pool(name="px1", bufs=2, space="PSUM") as px1,
            tc.tile_pool(name="pp2", bufs=1, space="PSUM") as pp2,
            tc.tile_pool(name="ptr", bufs=1, space="PSUM") as ptr,
            tc.tile_pool(name="dram", bufs=1, space="DRAM") as dram,
        ):
            ixev_sb = constp.tile([128, tot_ev // 16], i16)
            ixod_sb = constp.tile([128, tot_od // 16], i16)
            disr_sb = constp.tile([64, NPP], f32)
            disr2_sb = constp.tile([64, NPP], bf16)
            dinv_sb = constp.tile([1, NPP], f32)
            w1_sb = constp.tile([DH, DH], f32)
            w2_sb = constp.tile([DH, DOUT], bf16)
            b1_sb = constp.tile([1, DH], f32)
            b2_sb = constp.tile([64, 1], f32)
            identb_sb = constp.tile([64, 64], bf16)
            identf_sb = constp.tile([64, 64], f32)
            ident2_sb = constp.tile([DH, 64], f32)
            for sb, dr in [(ixev_sb, ixev_d), (ixod_sb, ixod_d),
                           (disr_sb, disr_d), (disr2_sb, disr2_d),
                           (dinv_sb, dinv_d), (w1_sb, w1_d), (w2_sb, w2_d),
                           (b1_sb, b1_d), (b2_sb, b2_d),
                           (identb_sb, identb_d), (identf_sb, identf_d),
                           (ident2_sb, ident2_d)]:
                nc.sync.dma_start(sb[:], dr[:])

            abl = os.environ.get("GCN_ABL", "")
            for _rep in range(int(os.environ.get("GCN_REPEAT", "1"))):
                part = dram.tile([NPP, 64], bf16, tag="part", bufs=2)
                table = dram.tile([NROWS, 64], bf16, addr_space="Shared",
                                  tag="table", bufs=2)

                for lidx in range(2):
                    tabv = (xt_d[:, :] if lidx == 0 else
                            table[:, :].rearrange("(n k) f -> n (k f)", k=2))
                    for g, (t0, nt) in enumerate(groups):
                        nd = nt * 128
                        rev, rod = int(R_ev[g]), int(R_od[g])
                        nev, nod = nd * rev, nd * rod
                        gev = gevp.tile([128, nev], bf16, tag="gev")
                        if abl != "nog":
                            nc.gpsimd.dma_gather(
                                gev[:].rearrange("p (one n) -> p one n",
                                                 one=1),
                                tabv,
                                ixev_sb[:, ds(int(off_ev[g]) // 16,
                                              nev // 16)],
                                num_idxs=nev, num_idxs_reg=nev,
                                elem_size=128, transpose=True,
                                single_packet=False)
                        god = godp.tile([128, nod], bf16, tag="god")
                        if abl != "nog":
                            nc.gpsimd.dma_gather(
                                god[:].rearrange("p (one n) -> p one n",
                                                 one=1),
                                tabv,
                                ixod_sb[:, ds(int(off_od[g]) // 16,
                                              nod // 16)],
                                num_idxs=nod, num_idxs_reg=nod,
                                elem_size=128, transpose=True,
                                single_packet=False)
                        r12 = redp.tile([128, nd], f32, tag="r12")
                        nc.vector.tensor_reduce(
                            r12[0:64, :],
                            gev[0:64, :].rearrange("p (d r) -> p d r", r=rev),
                            axis=AX.X, op=ALU.add)
                        nc.vector.tensor_reduce(
                            r12[64:128, :],
                            god[64:128, :].rearrange("p (d r) -> p d r",
                                                     r=rod),
                            axis=AX.X, op=ALU.add)
                        col0 = t0 * 128
                        if abl == "gonly":
                            if lidx == 1:
                                ost = stgp.tile([64, 64], f32, tag="ostg")
                                nc.scalar.activation(
                                    ost[:], r12[0:64, 0:64], AF.Copy,
                                    bias=0.0)
                                nc.sync.dma_start(
                                    out_d[ds(col0, 64), :], ost[:])
                            continue
                        if lidx == 0:
                            x1p = px1.tile([DH, nd], f32, tag="x1p")
                            nc.tensor.matmul(x1p[:], w1_sb[:], r12[:],
                                             start=True, stop=False)
                            nc.tensor.matmul(
                                x1p[:], b1_sb[:],
                                dinv_sb[:, ds(col0, nd)],
                                start=False, stop=True)
                            x1sb = work.tile([DH, nd], bf16, tag="x1sb")
                            nc.scalar.activation(x1sb[:], x1p[:], AF.Lrelu,
                                                 bias=0.0, alpha=NEG_SLOPE)
                            p2p = pp2.tile([64, nd], f32, tag="p2p")
                            nc.tensor.matmul(p2p[:], w2_sb[:], x1sb[:],
                                             start=True, stop=True)
                            pts = work.tile([64, nd], bf16, tag="pts")
                            nc.vector.tensor_tensor(
                                pts[:], p2p[:], disr2_sb[:, ds(col0, nd)],
                                op=ALU.mult)
                            pT = ptr.tile([128, nt * 64], bf16, tag="pT")
                            for q in range(nt):
                                nc.tensor.transpose(
                                    pT[:, q * 64:(q + 1) * 64],
                                    pts[:, q * 128:(q + 1) * 128],
                                    identb_sb[:])
                            stage = stgp.tile([128, nt * 64], bf16,
                                              tag="stage")
                            nc.scalar.activation(stage[:], pT[:], AF.Copy,
                                                 bias=0.0)
                            nc.sync.dma_start(
                                part[ds(col0, nd), :].rearrange(
                                    "(q p) f -> p q f", p=128),
                                stage[:].rearrange("p (q f) -> p q f", f=64))
                        else:
                            a2p = pp2.tile([64, nd], f32, tag="a2p")
                            nc.tensor.matmul(a2p[:], ident2_sb[:], r12[:],
                                             start=True, stop=True)
                            aggs = work.tile([64, nd], f32, tag="aggs")
                            nc.vector.tensor_tensor(
                                aggs[:], a2p[:], disr_sb[:, ds(col0, nd)],
                                op=ALU.mult)
                            osbT = work.tile([64, nd], f32, tag="osbT")
                            nc.scalar.activation(osbT[:], aggs[:], AF.Lrelu,
                                                 bias=b2_sb[:, 0:1],
                                                 alpha=NEG_SLOPE)
                            oT = ptr.tile([128, nt * 64], f32, tag="oT")
                            for q in range(nt):
                                nc.tensor.transpose(
                                    oT[:, q * 64:(q + 1) * 64],
                                    osbT[:, q * 128:(q + 1) * 128],
                                    identf_sb[:])
                            ost = stgp.tile([128, nt * 64], f32, tag="ost")
                            nc.scalar.activation(ost[:], oT[:], AF.Copy,
                                                 bias=0.0)
                            nc.sync.dma_start(
                                out_d[ds(col0, nd), :].rearrange(
                                    "(q p) f -> p q f", p=128),
                                ost[:].rearrange("p (q f) -> p q f", f=64))
                    if lidx == 0 and abl != "gonly":
                        if os.environ.get("GCN_NOAG", "0") == "1":
                            nc.sync.dma_start(table[0:NPP, :], part[:, :])
                        else:
                            nc.gpsimd.collective_compute(
                                "AllGather", mybir.AluOpType.bypass,
                                replica_groups=[list(range(P_CORES))],
                                ins=[part.opt()], outs=[table.opt()],
                            )

    nc.compile()
    return nc


def _make_in_maps(inputs, W1, b1, W2, b2, prep):
    import ml_dtypes
    dis = prep["dis"]
    rank_of = prep["rank_of"]
    x32 = np.asarray(inputs, np.float32) * dis[:, None]   # dis_s * x_s
    xt = np.zeros((NROWS, 64), dtype=np.float32)
    k = np.arange(N)
    rows = (k % P_CORES) * NPP + (k // P_CORES)
    xt[rows, :] = x32[rank_of]
    xt = xt.reshape(NPAIR, 128).astype(ml_dtypes.bfloat16)
    ident = np.eye(64, dtype=np.float32)
    W1np = np.asarray(W1, np.float32)
    w1dup = np.concatenate([W1np, W1np], axis=0)          # [128, 128]
    ident2 = np.concatenate([ident, ident], axis=0)       # [128, 64]
    in_maps = []
    for c in range(P_CORES):
        in_maps.append({
            "xt": xt,
            "ixev": prep["idx_ev"][c],
            "ixod": prep["idx_od"][c],
            "disr": prep["disrow"][c],
            "disr2": prep["disrow2"][c].astype(ml_dtypes.bfloat16),
            "dinv": prep["dinv"][c],
            "w1d": w1dup,
            "w2b": np.asarray(W2, np.float32).astype(ml_dtypes.bfloat16),
            "b1r": np.asarray(b1, np.float32).reshape(1, DH),
            "b2c": np.asarray(b2, np.float32).reshape(64, 1),
            "identb": ident.astype(ml_dtypes.bfloat16),
            "identf": ident,
            "ident2": ident2,
        })
    return in_maps


_CACHE = {}


def kernel(inputs, edge_index, W1, b1, W2, b2, _trace=False, _results_box=None):
    from concourse.bass_utils import run_bass_kernel_spmd

    edge_index = np.asarray(edge_index)
    key = hashlib.sha1(edge_index.tobytes()).hexdigest()
    key += ":r%s:n%s:a%s" % (os.environ.get("GCN_REPEAT", "1"),
                             os.environ.get("GCN_NOAG", "0"),
                             os.environ.get("GCN_ABL", ""))
    if key not in _CACHE:
        prep = _prep(edge_index)
        nc = _build_nc(prep)
        _CACHE[key] = (prep, nc)
    prep, nc = _CACHE[key]
    in_maps = _make_in_maps(inputs, W1, b1, W2, b2, prep)
    res = run_bass_kernel_spmd(
        nc, in_maps, core_ids=list(range(P_CORES)), trace=_trace,
    )
    if _results_box is not None:
        _results_box.append(res)
    outp = np.empty((N, DOUT), dtype=np.float32)
    rank_of = prep["rank_of"]
    for c in range(P_CORES):
        o = res.results[c]["outp"][:NP]
        ranks = np.arange(NP) * P_CORES + c
        outp[rank_of[ranks]] = o
    return outp


# revision 12
# speedup vs baseline: 7.1883x; 1.0292x over previous
"""2-layer GCN (PyG GCNConv x2 + leaky_relu) on 8 Trainium2 NeuronCores.

v4 strategy (pair-packed gather + segment-reduce):
  - Nodes ranked by degree, dealt round-robin across cores (rank k ->
    core k%8, local pos k//8): every core's tile t holds nodes of nearly
    identical degree, so slot capacities are tight and SPMD-uniform.
  - The node table is pair-packed: DRAM row m (256B) holds features of
    gather-rows 2m and 2m+1 (64 bf16 each). dma_gather with idx=row//2
    (always < 25088, int16-safe) lands pairs feature-major: partitions
    0..63 = even row, 64..127 = odd row. Per dst, in-edges (+ self loop)
    are split by src-row parity into even/odd slot grids; one gather per
    half per tile-group, one DVE tensor_reduce per half writing disjoint
    partition halves of a single [128, nd] accumulator.
  - The even/odd halves are merged by the tail matmul itself: W1 is
    duplicated across partitions 0..63/64..127 so the K=128 contraction
    sums both halves (layer 2 uses a stacked-identity matmul instead).
  - Tail per group (<=4 tiles, 512 dst columns/instruction): W1 matmul +
    rank-1 b1*dinv into PSUM, ACT lrelu, W2 matmul, DVE dis^2 scale,
    per-tile PE transpose, ACT copy, one strided store. Layer-2 table is
    built by a 64-col AllGather of the per-core part buffers (the pair
    view is just an AP reshape - no expand pass).
  - Group sizes adapt to per-tile slot caps so the worst gather buffers
    stay small enough for bufs=3 buffering (deep DMA pipeline).

Self-contained: hardcodes shapes; compiles on first call keyed by edge hash.
"""

import os
import hashlib
import sys

import numpy as np

sys.path.insert(0, "/opt/trn_rl_repo")

# ---- problem constants ----
N, E = 50000, 800000
DIN, DH, DOUT = 64, 128, 64
P_CORES = 8
NP = N // P_CORES            # 6250 real nodes per core
NT = 49                      # dst tiles per core
NPP = NT * 128               # 6272 padded rows per core
NROWS = P_CORES * NPP        # 50176 table rows
NPAIR = NROWS // 2           # 25088 pair rows
GSZ = 4                      # max tiles per group
CAP_SLOTS = 7168             # max slots per group-half buffer (14KB/part)
NEG_SLOPE = 0.01


def _prep(edge_index: np.ndarray):
    src = np.asarray(edge_index[0], dtype=np.int64)
    dst = np.asarray(edge_index[1], dtype=np.int64)

    deg = (np.bincount(dst, minlength=N) + 1).astype(np.float32)
    dis = (1.0 / np.sqrt(deg)).astype(np.float32)

    # degree-descending rank -> (core, local pos)
    rank_of = np.argsort(-deg, kind="stable")      # rank -> orig node
    newpos = np.empty(N, dtype=np.int64)           # orig -> rank
    newpos[rank_of] = np.arange(N)
    core_of = newpos % P_CORES
    loc_of = newpos // P_CORES
    grow_of = core_of * NPP + loc_of               # orig -> gather row

    # edge slot lists (self loop appended for every node)
    s_all = np.concatenate([src, np.arange(N, dtype=np.int64)])
    d_all = np.concatenate([dst, np.arange(N, dtype=np.int64)])
    gs = grow_of[s_all]
    dcore = core_of[d_all]
    dloc = loc_of[d_all]
    is_ev = (gs % 2) == 0

    key = ((dcore * NPP + dloc) * 2 + (~is_ev).astype(np.int64))
    order = np.argsort(key, kind="stable")
    ks = key[order]
    first = np.ones(len(ks), dtype=bool)
    first[1:] = ks[1:] != ks[:-1]
    starts = np.flatnonzero(first)
    run_id = np.cumsum(first) - 1
    rnk = np.arange(len(ks)) - starts[run_id]

    gso = gs[order]
    dco = dcore[order]
    dlo = dloc[order]
    evo = is_ev[order]

    cnt_ev = np.zeros((P_CORES, NPP), dtype=np.int64)
    cnt_od = np.zeros((P_CORES, NPP), dtype=np.int64)
    np.add.at(cnt_ev, (dcore[is_ev], dloc[is_ev]), 1)
    np.add.at(cnt_od, (dcore[~is_ev], dloc[~is_ev]), 1)

    # per-tile capacities (max across cores)
    Rt_ev = np.maximum(1, cnt_ev.reshape(P_CORES, NT, 128).max(axis=(0, 2)))
    Rt_od = np.maximum(1, cnt_od.reshape(P_CORES, NT, 128).max(axis=(0, 2)))

    # adaptive grouping: <= GSZ tiles and <= CAP_SLOTS per half
    groups = []                                    # list of (t0, nt)
    t = 0
    while t < NT:
        nt = 1
        while (t + nt < NT and nt < GSZ
               and (nt + 1) * 128 * max(Rt_ev[t:t + nt + 1]) <= CAP_SLOTS
               and (nt + 1) * 128 * max(Rt_od[t:t + nt + 1]) <= CAP_SLOTS):
            nt += 1
        groups.append((t, nt))
        t += nt
    R_ev = np.array([max(Rt_ev[t0:t0 + nt]) for t0, nt in groups])
    R_od = np.array([max(Rt_od[t0:t0 + nt]) for t0, nt in groups])

    n_ev = [int(nt * 128 * R_ev[g]) for g, (t0, nt) in enumerate(groups)]
    n_od = [int(nt * 128 * R_od[g]) for g, (t0, nt) in enumerate(groups)]
    off_ev = np.concatenate([[0], np.cumsum(n_ev)]).astype(np.int64)
    off_od = np.concatenate([[0], np.cumsum(n_od)]).astype(np.int64)
    tot_ev, tot_od = int(off_ev[-1]), int(off_od[-1])

    DUMMY = NP // 2        # pair row 3125 = rows (6250, 6251), core0 pads
    idx_ev = np.full((P_CORES, tot_ev), DUMMY, dtype=np.int64)
    idx_od = np.full((P_CORES, tot_od), DUMMY, dtype=np.int64)

    # map each loc to its group and position within the group
    g_of_tile = np.empty(NT, dtype=np.int64)
    qoff_of_tile = np.empty(NT, dtype=np.int64)    # (q*128) offset in group
    for g, (t0, nt) in enumerate(groups):
        g_of_tile[t0:t0 + nt] = g
        qoff_of_tile[t0:t0 + nt] = np.arange(nt) * 128
    tile_of = dlo // 128
    gg = g_of_tile[tile_of]
    qp = qoff_of_tile[tile_of] + (dlo % 128)

    sel = evo
    j = off_ev[gg[sel]] + qp[sel] * R_ev[gg[sel]] + rnk[sel]
    idx_ev[dco[sel], j] = gso[sel] // 2
    sel = ~evo
    j = off_od[gg[sel]] + qp[sel] * R_od[gg[sel]] + rnk[sel]
    idx_od[dco[sel], j] = gso[sel] // 2

    def wrap16(a, tot):
        cols = tot // 16
        t = np.zeros((P_CORES, 128, cols), dtype=np.int16)
        v = a.astype(np.int16).reshape(P_CORES, cols, 16)
        for rs in range(0, 128, 16):
            t[:, rs:rs + 16, :] = v.transpose(0, 2, 1)
        return t

    assert tot_ev % 16 == 0 and tot_od % 16 == 0
    idx_ev_t = wrap16(idx_ev, tot_ev)
    idx_od_t = wrap16(idx_od, tot_od)

    dis_loc = np.zeros((P_CORES, NPP), dtype=np.float32)
    for c in range(P_CORES):
        n_ids = rank_of[c::P_CORES]
        dis_loc[c, :len(n_ids)] = dis[n_ids]
    disrow = np.broadcast_to(dis_loc[:, None, :], (P_CORES, 64, NPP)).copy()
    disrow2 = (disrow * disrow).copy()
    dinv = np.where(dis_loc > 0, 1.0 / np.maximum(dis_loc, 1e-9), 0.0)
    dinv = dinv.reshape(P_CORES, 1, NPP).astype(np.float32)

    return dict(dis=dis, rank_of=rank_of,
                groups=groups, R_ev=R_ev, R_od=R_od,
                off_ev=off_ev, off_od=off_od, tot_ev=tot_ev, tot_od=tot_od,
                idx_ev=idx_ev_t, idx_od=idx_od_t,
                disrow=disrow, disrow2=disrow2, dinv=dinv)


# ---------------------------------------------------------------------------
# Bass kernel
# ---------------------------------------------------------------------------

def _build_nc(prep):
    import concourse.bass as bass
    import concourse.bacc as bacc
    import concourse.tile as tile
    from concourse import mybir

    f32 = mybir.dt.float32
    bf16 = mybir.dt.bfloat16
    i16 = mybir.dt.int16
    AF = mybir.ActivationFunctionType
    ALU = mybir.AluOpType
    AX = mybir.AxisListType
    ds = bass.ds

    groups = prep["groups"]
    R_ev, R_od = prep["R_ev"], prep["R_od"]
    off_ev, off_od = prep["off_ev"], prep["off_od"]
    tot_ev, tot_od = prep["tot_ev"], prep["tot_od"]

    nc = bacc.Bacc(
        "TRN2", target_bir_lowering=False, debug=False,
        enable_asserts=False, num_devices=P_CORES,
    )

    xt_d = nc.dram_tensor("xt", [NPAIR, 128], bf16, kind="ExternalInput")
    ixev_d = nc.dram_tensor("ixev", [128, tot_ev // 16], i16,
                            kind="ExternalInput")
    ixod_d = nc.dram_tensor("ixod", [128, tot_od // 16], i16,
                            kind="ExternalInput")
    disr_d = nc.dram_tensor("disr", [64, NPP], f32, kind="ExternalInput")
    disr2_d = nc.dram_tensor("disr2", [64, NPP], bf16, kind="ExternalInput")
    dinv_d = nc.dram_tensor("dinv", [1, NPP], f32, kind="ExternalInput")
    w1_d = nc.dram_tensor("w1d", [DH, DH], f32, kind="ExternalInput")
    w2_d = nc.dram_tensor("w2b", [DH, DOUT], bf16, kind="ExternalInput")
    b1_d = nc.dram_tensor("b1r", [1, DH], f32, kind="ExternalInput")
    b2_d = nc.dram_tensor("b2c", [64, 1], f32, kind="ExternalInput")
    identb_d = nc.dram_tensor("identb", [64, 64], bf16, kind="ExternalInput")
    identf_d = nc.dram_tensor("identf", [64, 64], f32, kind="ExternalInput")
    ident2_d = nc.dram_tensor("ident2", [DH, 64], f32, kind="ExternalInput")
    out_d = nc.dram_tensor("outp", [64, NPP], f32, kind="ExternalOutput")

    with tile.TileContext(nc) as tc:
        with (
            tc.tile_pool(name="const", bufs=1) as constp,
            tc.tile_pool(name="gev", bufs=3) as gevp,
            tc.tile_pool(name="god", bufs=3) as godp,
            tc.tile_pool(name="red", bufs=3) as redp,
            tc.tile_pool(name="wk", bufs=2) as work,
            tc.tile_pool(name="stg", bufs=2) as stgp,
            tc.tile_pool(name="px1", bufs=2, space="PSUM") as px1,
            tc.tile_pool(name="pp2", bufs=1, space="PSUM") as pp2,
            tc.tile_pool(name="ptr", bufs=1, space="PSUM") as ptr,
            tc.tile_pool(name="dram", bufs=1, space="DRAM") as dram,
        ):
            ixev_sb = constp.tile([128, tot_ev // 16], i16)
            ixod_sb = constp.tile([128, tot_od // 16], i16)
            disr_sb = constp.tile([64, NPP], f32)
            disr2_sb = constp.tile([64, NPP], bf16)
            dinv_sb = constp.tile([1, NPP], f32)
            w1_sb = constp.tile([DH, DH], f32)
            w2_sb = constp.tile([DH, DOUT], bf16)
            b1_sb = constp.tile([1, DH], f32)
            b2_sb = constp.tile([64, 1], f32)
            identb_sb = constp.tile([64, 64], bf16)
            identf_sb = constp.tile([64, 64], f32)
            ident2_sb = constp.tile([DH, 64], f32)
            for sb, dr in [(ixev_sb, ixev_d), (ixod_sb, ixod_d),
                           (disr_sb, disr_d), (disr2_sb, disr2_d),
                           (dinv_sb, dinv_d), (w1_sb, w1_d), (w2_sb, w2_d),
                           (b1_sb, b1_d), (b2_sb, b2_d),
                           (identb_sb, identb_d), (identf_sb, identf_d),
                           (ident2_sb, ident2_d)]:
                nc.sync.dma_start(sb[:], dr[:])

            abl = os.environ.get("GCN_ABL", "")
            for _rep in range(int(os.environ.get("GCN_REPEAT", "1"))):
                part = dram.tile([NPP, 64], bf16, tag="part", bufs=2)
                table = dram.tile([NROWS, 64], bf16, addr_space="Shared",
                                  tag="table", bufs=2)

                for lidx in range(2):
                    tabv = (xt_d[:, :] if lidx == 0 else
                            table[:, :].rearrange("(n k) f -> n (k f)", k=2))
                    for g, (t0, nt) in enumerate(groups):
                        nd = nt * 128
                        rev, rod = int(R_ev[g]), int(R_od[g])
                        nev, nod = nd * rev, nd * rod
                        gev = gevp.tile([128, nev], bf16, tag="gev")
                        if abl != "nog":
                            nc.gpsimd.dma_gather(
                                gev[:].rearrange("p (one n) -> p one n",
                                                 one=1),
                                tabv,
                                ixev_sb[:, ds(int(off_ev[g]) // 16,
                                              nev // 16)],
                                num_idxs=nev, num_idxs_reg=nev,
                                elem_size=128, transpose=True,
                                single_packet=False)
                        god = godp.tile([128, nod], bf16, tag="god")
                        if abl != "nog":
                            nc.gpsimd.dma_gather(
                                god[:].rearrange("p (one n) -> p one n",
                                                 one=1),
                                tabv,
                                ixod_sb[:, ds(int(off_od[g]) // 16,
                                              nod // 16)],
                                num_idxs=nod, num_idxs_reg=nod,
                                elem_size=128, transpose=True,
                                single_packet=False)
                        r12 = redp.tile([128, nd], f32, tag="r12")
                        nc.vector.tensor_reduce(
                            r12[0:64, :],
                            gev[0:64, :].rearrange("p (d r) -> p d r", r=rev),
                            axis=AX.X, op=ALU.add)
                        nc.vector.tensor_reduce(
                            r12[64:128, :],
                            god[64:128, :].rearrange("p (d r) -> p d r",
                                                     r=rod),
                            axis=AX.X, op=ALU.add)
                        col0 = t0 * 128
                        if abl == "gonly":
                            if lidx == 1:
                                ost = stgp.tile([64, 64], f32, tag="ostg")
                                nc.scalar.activation(
                                    ost[:], r12[0:64, 0:64], AF.Copy,
                                    bias=0.0)
                                nc.sync.dma_start(
                                    out_d[:, ds(col0, 64)], ost[:])
                            continue
                        if lidx == 0:
                            x1p = px1.tile([DH, nd], f32, tag="x1p")
                            nc.tensor.matmul(x1p[:], w1_sb[:], r12[:],
                                             start=True, stop=False)
                            nc.tensor.matmul(
                                x1p[:], b1_sb[:],
                                dinv_sb[:, ds(col0, nd)],
                                start=False, stop=True)
                            x1sb = work.tile([DH, nd], bf16, tag="x1sb")
                            nc.scalar.activation(x1sb[:], x1p[:], AF.Lrelu,
                                                 bias=0.0, alpha=NEG_SLOPE)
                            p2p = pp2.tile([64, nd], f32, tag="p2p")
                            nc.tensor.matmul(p2p[:], w2_sb[:], x1sb[:],
                                             start=True, stop=True)
                            pts = work.tile([64, nd], bf16, tag="pts")
                            nc.vector.tensor_tensor(
                                pts[:], p2p[:], disr2_sb[:, ds(col0, nd)],
                                op=ALU.mult)
                            pT = ptr.tile([128, nt * 64], bf16, tag="pT")
                            for q in range(nt):
                                nc.tensor.transpose(
                                    pT[:, q * 64:(q + 1) * 64],
                                    pts[:, q * 128:(q + 1) * 128],
                                    identb_sb[:])
                            stage = stgp.tile([128, nt * 64], bf16,
                                              tag="stage")
                            nc.scalar.activation(stage[:], pT[:], AF.Copy,
                                                 bias=0.0)
                            nc.sync.dma_start(
                                part[ds(col0, nd), :].rearrange(
                                    "(q p) f -> p q f", p=128),
                                stage[:].rearrange("p (q f) -> p q f", f=64))
                        else:
                            a2p = pp2.tile([64, nd], f32, tag="a2p")
                            nc.tensor.matmul(a2p[:], ident2_sb[:], r12[:],
                                             start=True, stop=True)
                            aggs = work.tile([64, nd], f32, tag="aggs")
                            nc.vector.tensor_tensor(
                                aggs[:], a2p[:], disr_sb[:, ds(col0, nd)],
                                op=ALU.mult)
                            osbT = work.tile([64, nd], f32, tag="osbT")
                            nc.scalar.activation(osbT[:], aggs[:], AF.Lrelu,
                                                 bias=b2_sb[:, 0:1],
                                                 alpha=NEG_SLOPE)
                            nc.sync.dma_start(
                                out_d[:, ds(col0, nd)], osbT[:])
                    if lidx == 0 and abl != "gonly":
                        if os.environ.get("GCN_NOAG", "0") == "1":
                            pass
                        else:
                            nc.gpsimd.collective_compute(
                                "AllGather", mybir.AluOpType.bypass,
                                replica_groups=[list(range(P_CORES))],
                                ins=[part.opt()], outs=[table.opt()],
                            )

    nc.compile()
    return nc


def _make_in_maps(inputs, W1, b1, W2, b2, prep):
    import ml_dtypes
    dis = prep["dis"]
    rank_of = prep["rank_of"]
    x32 = np.asarray(inputs, np.float32) * dis[:, None]   # dis_s * x_s
    xt = np.zeros((NROWS, 64), dtype=np.float32)
    k = np.arange(N)
    rows = (k % P_CORES) * NPP + (k // P_CORES)
    xt[rows, :] = x32[rank_of]
    xt = xt.reshape(NPAIR, 128).astype(ml_dtypes.bfloat16)
    ident = np.eye(64, dtype=np.float32)
    W1np = np.asarray(W1, np.float32)
    w1dup = np.concatenate([W1np, W1np], axis=0)          # [128, 128]
    ident2 = np.concatenate([ident, ident], axis=0)       # [128, 64]
    in_maps = []
    for c in range(P_CORES):
        in_maps.append({
            "xt": xt,
            "ixev": prep["idx_ev"][c],
            "ixod": prep["idx_od"][c],
            "disr": prep["disrow"][c],
            "disr2": prep["disrow2"][c].astype(ml_dtypes.bfloat16),
            "dinv": prep["dinv"][c],
            "w1d": w1dup,
            "w2b": np.asarray(W2, np.float32).astype(ml_dtypes.bfloat16),
            "b1r": np.asarray(b1, np.float32).reshape(1, DH),
            "b2c": np.asarray(b2, np.float32).reshape(64, 1),
            "identb": ident.astype(ml_dtypes.bfloat16),
            "identf": ident,
            "ident2": ident2,
        })
    return in_maps


_CACHE = {}


def kernel(inputs, edge_index, W1, b1, W2, b2, _trace=False, _results_box=None):
    from concourse.bass_utils import run_bass_kernel_spmd

    edge_index = np.asarray(edge_index)
    key = hashlib.sha1(edge_index.tobytes()).hexdigest()
    key += ":r%s:n%s:a%s" % (os.environ.get("GCN_REPEAT", "1"),
                             os.environ.get("GCN_NOAG", "0"),
                             os.environ.get("GCN_ABL", ""))
    if key not in _CACHE:
        prep = _prep(edge_index)
        nc = _build_nc(prep)
        _CACHE[key] = (prep, nc)
    prep, nc = _CACHE[key]
    in_maps = _make_in_maps(inputs, W1, b1, W2, b2, prep)
    res = run_bass_kernel_spmd(
        nc, in_maps, core_ids=list(range(P_CORES)), trace=_trace,
    )
    if _results_box is not None:
        _results_box.append(res)
    outp = np.empty((N, DOUT), dtype=np.float32)
    rank_of = prep["rank_of"]
    for c in range(P_CORES):
        o = res.results[c]["outp"][:, :NP]           # [64, NP] feature-major
        ranks = np.arange(NP) * P_CORES + c
        outp[rank_of[ranks]] = o.T
    return outp
